# revision 1
# baseline (speedup 1.0000x reference)
"""Multi-head attention (B=2, S=2048, D=1024, H=16) on 8 TRN2 NeuronCores.

Sharding: core c handles batch b = c//4 and head group hg = c%4 (4 heads,
256 features f0 = hg*256). Each core computes Q/K/V projections for its
feature slice, attention for its 4 heads, and a partial output projection
y_partial = attnout @ Wo[:, f0:f0+256].T (emitted in fp16). Host sums the 4
partials per batch and adds bo.

Single fused software-pipelined loop, engineered so the tensor engine's
instruction stream is dense (TRN2 PE DVFS only reaches 2.4GHz after ~3us of
continuous execution):
 - K/V projections are JIT-streamed per k-tile-group inside q-chunk 0's
   attention loop; Q projections prefetched one (qc,pr) iteration ahead;
   out-projection of chunk qc deferred into chunk qc+1's loop. The PE
   therefore always has independent matmuls queued while softmax exp runs.
 - scores per head pair are issued back-to-back as K=64 matmuls on PE row
   groups 0:63 / 64:127 (tile_position auto-derived from base partition) so
   the two matmuls execute concurrently on the array.
 - softmax exp: scalar-engine Exp on [128, 2, 512] PSUM spans (two heads per
   instruction), with a fraction of k-tiles offloaded to DVE/Pool via a
   one-instruction fp16 Schraudolph exp (t = round(A*s + B) written as int16,
   bit-reinterpreted as fp16; constant-offset error is common-mode across k
   and cancels in the softmax normalization).
 - normalization: row sums ride along in the AV matmul (ones column in the
   augmented V); psav is drained early to SBUF (frees the PSUM bank), sums
   broadcast across partitions via two K=1 ones-matmuls into one PSUM tile,
   a single reciprocal_approx_fast over all 128 partitions, then two
   multiplies produce attnout.T in fp16.

All matmuls fp16 (PSUM accumulation fp32); elementwise fp32 on DVE/Pool.
"""
import numpy as np

import concourse.bass as bass
import concourse.mybir as mybir
import concourse.tile as tile
from concourse import bacc
from concourse import bass_utils

F32 = mybir.dt.float32
F16 = mybir.dt.float16
I16 = mybir.dt.int16
EXP = mybir.ActivationFunctionType.Exp
ADD = mybir.AluOpType.add
MULT = mybir.AluOpType.mult

B, S, D, H = 2, 2048, 1024, 16
HD = D // H          # 64
E = 256              # local features per core (4 heads)
QC = 512             # q-chunk size for the attention phase
N_QC = S // QC       # 4
N_KT = S // 128      # 16 k-tiles
KD = D // 128        # 8 contraction tiles for the projections

# Schraudolph fp16 exp: bits = round(x * 1024/ln2 + 15360 + C); the 0.125
# logit scale is folded into A. C=-44 minimizes max rel err (~3.1%); the
# constant-offset component cancels in the softmax normalization.
SCHR_A = 0.125 * 1024.0 / float(np.log(2.0))
SCHR_B = 15360.0 - 44.0
# k-tiles offloaded off the scalar engine per (qc, pr) iteration for qc >= 1
# (alternating DVE / Pool).
OFF_DVE = (3, 7, 11, 13)
OFF_POOL = ()


def build():
    nc = bacc.Bacc("TRN2", target_bir_lowering=False, debug=False, num_devices=8)

    xT = nc.dram_tensor("xT", [D, S], F16, kind="ExternalInput").ap()
    # wkq = [wkT | wqT] concatenated host-side so one DMA per chunk loads both
    wkq = nc.dram_tensor("wkq", [D, 2 * E], F16, kind="ExternalInput").ap()
    wvT = nc.dram_tensor("wvT", [D, E], F16, kind="ExternalInput").ap()
    woT = nc.dram_tensor("woT", [E, D], F16, kind="ExternalInput").ap()
    bq2 = nc.dram_tensor("bq2", [128, 2], F32, kind="ExternalInput").ap()
    bk2 = nc.dram_tensor("bk2", [128, 2], F32, kind="ExternalInput").ap()
    bvb = nc.dram_tensor("bvb", [128, E], F32, kind="ExternalInput").ap()
    # full V_aug constant patterns (ones/zeros; V columns overwritten by the
    # V projection): even = ones at col 64, odd = ones at col 0
    vce = nc.dram_tensor("vce", [128, N_KT, 128], F16, kind="ExternalInput").ap()
    vco = nc.dram_tensor("vco", [128, N_KT, 128], F16, kind="ExternalInput").ap()
    ones128 = nc.dram_tensor("ones128", [1, 128], F16, kind="ExternalInput").ap()

    y = nc.dram_tensor("y", [S, D], F16, kind="ExternalOutput").ap()

    with tile.TileContext(nc) as tc:
        with (
            tc.tile_pool(name="pool", bufs=1) as pp,
            tc.tile_pool(name="pexp_pool", bufs=8) as pxp,
            tc.tile_pool(name="work", bufs=4) as wk,
            tc.tile_pool(name="rpool", bufs=2) as rp,
            tc.tile_pool(name="ypool", bufs=4) as yp,
            tc.tile_pool(name="ps_s", bufs=2, space="PSUM") as ps_s,
            tc.tile_pool(name="ps_av", bufs=2, space="PSUM") as ps_av,
            tc.tile_pool(name="ps_misc", bufs=2, space="PSUM") as ps_misc,
        ):
            # ---------------- persistent tiles ----------------
            woT_sb = pp.tile([128, 2, D], F16)
            bvb_sb = pp.tile([128, E], F32)
            ones_sb = pp.tile([128, 128], F16)     # partitions 0 and 64 used
            bq_sb = pp.tile([128, 2], F32)
            bk_sb = pp.tile([128, 2], F32)
            QT_sb = pp.tile([128, 2, S], F16)
            KT_sb = pp.tile([128, 2, S], F16)
            OT_sb = pp.tile([128, 2, S], F16)
            xT_sb = pp.tile([128, KD, S], F16)
            wkq_sb = pp.tile([128, KD, 2 * E], F16)
            wv_sb = pp.tile([128, KD, E], F16)
            # V_aug per pair (128 cols each so the matmul dst is a full
            # 128-partition AP):
            #   even head: [*, kt, 0:64]=V, col 64=1, cols 65:128=0
            #   odd head:  col 0=1, cols 1:64=0, [*, kt, 64:128]=V
            Ve_sb = [pp.tile([128, N_KT, 128], F16, name=f"ve{p}", tag=f"ve{p}")
                     for p in range(2)]
            Vo_sb = [pp.tile([128, N_KT, 128], F16, name=f"vo{p}", tag=f"vo{p}")
                     for p in range(2)]

            # ---------------- input DMAs ----------------
            # All transfers are 2-D [128, X] per-chunk DMAs (the fast DMA
            # path). The warmup is DMA-issue-rate-bound (~0.7us per issue per
            # queue), so K and Q weights are fused into one tensor (wkq) and
            # xT s-block pairs are fused into [128, 1024] transfers.
            # sync: xT blocks 0-1; scalar: wkq then xT blocks 2-3;
            # gpsimd: wv + V_aug constants + woT.
            for k in range(KD):
                nc.sync.dma_start(
                    xT_sb[:, k, 0:1024], xT[k * 128:(k + 1) * 128, 0:1024])
            for k in range(KD):
                nc.scalar.dma_start(
                    wkq_sb[:, k, :], wkq[k * 128:(k + 1) * 128, :])
            nc.scalar.dma_start(bk_sb[:], bk2)
            nc.scalar.dma_start(bq_sb[:], bq2)
            nc.scalar.dma_start(ones_sb[0:1, :], ones128)
            nc.scalar.dma_start(ones_sb[64:65, :], ones128)
            for k in range(KD):
                nc.scalar.dma_start(
                    xT_sb[:, k, 1024:2048], xT[k * 128:(k + 1) * 128, 1024:2048])
            for k in range(KD):
                nc.gpsimd.dma_start(wv_sb[:, k, :], wvT[k * 128:(k + 1) * 128, :])
            for pr in range(2):
                nc.gpsimd.dma_start(Ve_sb[pr][:], vce)
                nc.gpsimd.dma_start(Vo_sb[pr][:], vco)
            nc.gpsimd.dma_start(bvb_sb[:], bvb)
            for p in range(2):
                nc.gpsimd.dma_start(woT_sb[:, p, :], woT[p * 128:(p + 1) * 128, :])

            # ---------------- emit helpers ----------------
            def proj_chain(w0, b_sb, out_sb, ch, g):
                """[128, 512] projection chunk: out_sb[:, ch, g*512:...] =
                W_ch.T @ xT[:, g-block] + b (drained on DVE). w0 selects the
                K (0) or Q (E) half of the fused wkq weights."""
                ssl = slice(g * 512, (g + 1) * 512)
                ps = ps_misc.tile([128, 512], F32, tag="misc")
                for k in range(KD):
                    nc.tensor.matmul(
                        ps[:],
                        wkq_sb[:, k, w0 + ch * 128:w0 + (ch + 1) * 128],
                        xT_sb[:, k, ssl],
                        start=(k == 0), stop=(k == KD - 1))
                nc.vector.tensor_scalar(
                    out_sb[:, ch, ssl], ps[:], b_sb[:, ch:ch + 1], None, ADD)

            def kproj(ch, g):
                proj_chain(0, bk_sb, KT_sb, ch, g)

            def qproj(qc, ch):
                proj_chain(E, bq_sb, QT_sb, ch, qc)

            def vproj(st):
                """V for s-tile st (all 4 heads), scattered+biased into the
                augmented V tiles on Pool."""
                ps = ps_misc.tile([128, 512], F32, tag="misc")
                for k in range(KD):
                    nc.tensor.matmul(
                        ps[:, 0:E],
                        xT_sb[:, k, st * 128:(st + 1) * 128],
                        wv_sb[:, k, :],
                        start=(k == 0), stop=(k == KD - 1))
                for h in range(4):
                    pr, odd = h // 2, h % 2
                    dst = (Vo_sb[pr][:, st, 64:128] if odd
                           else Ve_sb[pr][:, st, 0:64])
                    nc.vector.tensor_tensor(
                        dst, ps[:, h * 64:(h + 1) * 64],
                        bvb_sb[:, h * 64:(h + 1) * 64], ADD)

            def outproj_unit(st, nch):
                """y[s-tile st, nch*512:...] = OT[:, :, ssl].T @ woT (both
                contraction chunks), drained to fp16 on Pool, DMA'd on sync."""
                ssl = slice(st * 128, (st + 1) * 128)
                psy = ps_misc.tile([128, 512], F32, tag="misc")
                for cc in range(2):
                    nc.tensor.matmul(
                        psy[:], OT_sb[:, cc, ssl],
                        woT_sb[:, cc, nch * 512:(nch + 1) * 512],
                        start=(cc == 0), stop=(cc == 1))
                y_sb = yp.tile([128, 512], F16, tag="y")
                nc.vector.tensor_copy(y_sb[:], psy[:])
                eng = nc.sync if nch == 0 else nc.gpsimd
                eng.dma_start(y[ssl, nch * 512:(nch + 1) * 512], y_sb[:])

            def attn_iter(qc, pr, fills):
                """One (q-chunk, head-pair) attention iteration. `fills` maps
                kt -> list of emit callables sprinkled into the loop to keep
                the PE stream dense. AV matmuls run AV_LAG k-tiles behind the
                score matmuls so the PE never waits on the exp latency.
                Returns a callable that emits the PE/DVE/Pool back half of the
                normalization (scheduled as a fill in the next iteration)."""
                qsl = slice(qc * QC, (qc + 1) * QC)
                av_e = ps_av.tile([128, QC], F32, tag="av")
                av_o = ps_av.tile([128, QC], F32, tag="av")
                pexps = {}
                AV_LAG = 3

                def emit_av(kt):
                    pexp = pexps.pop(kt)
                    nc.tensor.matmul(av_e[:], Ve_sb[pr][:, kt, :],
                                     pexp[:, 0, :],
                                     start=(kt == 0), stop=(kt == N_KT - 1))
                    nc.tensor.matmul(av_o[:], Vo_sb[pr][:, kt, :],
                                     pexp[:, 1, :],
                                     start=(kt == 0), stop=(kt == N_KT - 1))

                for kt in range(N_KT):
                    for f in fills.get(kt, ()):
                        f()
                    ksl = slice(kt * 128, (kt + 1) * 128)
                    # scores for the head pair: two K=64 matmuls on PE row
                    # groups 0:63 / 64:127, issued back-to-back so they run
                    # concurrently on the array.
                    ps = ps_s.tile([128, 2, QC], F32, tag="s")
                    nc.tensor.matmul(ps[:, 0, :], KT_sb[0:64, pr, ksl],
                                     QT_sb[0:64, pr, qsl])
                    nc.tensor.matmul(ps[:, 1, :], KT_sb[64:128, pr, ksl],
                                     QT_sb[64:128, pr, qsl])
                    pexp = pxp.tile([128, 2, QC], F16, tag="pexp")
                    pexps[kt] = pexp
                    if qc > 0 and kt in OFF_DVE:
                        nc.vector.tensor_scalar(
                            pexp[:].bitcast(I16), ps[:], SCHR_A, SCHR_B,
                            MULT, ADD)
                    else:
                        nc.scalar.activation(pexp[:], ps[:], EXP, scale=0.125)
                    if kt >= AV_LAG:
                        emit_av(kt - AV_LAG)
                for kt in range(N_KT - AV_LAG, N_KT):
                    emit_av(kt)
                # normalization, front half (DVE): sums rows to SBUF fp16
                # first (unblocks the broadcast matmuls), then drain psav to
                # SBUF (frees the PSUM banks for the next iteration).
                avsb_e = wk.tile([128, QC], F32, tag="avsb")
                avsb_o = wk.tile([128, QC], F32, tag="avsb")
                sums16 = wk.tile([128, QC], F16, tag="sums16")
                nc.vector.tensor_copy(sums16[64:65, :], av_e[64:65, :])
                nc.vector.tensor_copy(sums16[0:1, :], av_o[0:1, :])
                nc.vector.tensor_copy(avsb_e[:], av_e[:])
                nc.vector.tensor_copy(avsb_o[:], av_o[:])

                def finish_norm(fast=False):
                    # broadcast the raw sums (at partition 64 for even / 0
                    # for odd) via two K=1 ones-matmuls into one PSUM tile,
                    # one approx reciprocal over all 128 partitions, then
                    # scale on Pool (all-SBUF operands).
                    psbc = ps_misc.tile([128, 512], F32, tag="misc")
                    nc.tensor.matmul(psbc[0:64, :], ones_sb[64:65, 0:64],
                                     sums16[64:65, :])
                    nc.tensor.matmul(psbc[64:128, :], ones_sb[0:1, 64:128],
                                     sums16[0:1, :])
                    rec = rp.tile([128, QC], F32, tag="rec")
                    nc.vector.reciprocal_approx_fast(rec[:], psbc[:])
                    eng = nc.vector if (fast or pr == 1) else nc.gpsimd
                    eng.tensor_tensor(
                        OT_sb[0:64, pr, qsl], avsb_e[0:64, :], rec[0:64, :],
                        MULT)
                    eng.tensor_tensor(
                        OT_sb[64:128, pr, qsl], avsb_o[64:128, :],
                        rec[64:128, :], MULT)

                return finish_norm

            # ---------------- fused main loop ----------------
            # Warmup: K/Q first (their weights lead the scalar queue; scores
            # can then start early, warming up the scalar engine), V after
            # (its weights stream in on the gpsimd queue meanwhile).
            kproj(0, 0)
            qproj(0, 0)
            vproj(0)
            vproj(1)
            kproj(0, 1)
            vproj(2)
            vproj(3)
            vproj(4)
            vproj(5)

            pending_norm = None
            for qc in range(N_QC):
                for pr in range(2):
                    fills = {}
                    if pending_norm is not None:
                        fills.setdefault(1, []).append(pending_norm)
                    if qc == 0:
                        # JIT K/V projections. K chunk `pr` group g must
                        # precede scores kt=4g; V tile st must precede
                        # AV kt=st (prefetch distance 2 + AV lag).
                        if pr == 0:
                            for g in range(2, 4):
                                fills.setdefault(4 * (g - 1) + 2, []).append(
                                    (lambda g=g: kproj(0, g)))
                            for st in range(6, N_KT):
                                fills.setdefault(st - 6, []).append(
                                    (lambda st=st: vproj(st)))
                            fills.setdefault(12, []).append(lambda: qproj(0, 1))
                            fills.setdefault(13, []).append(lambda: kproj(1, 0))
                        else:
                            for g in range(1, 4):
                                fills.setdefault(4 * (g - 1) + 2, []).append(
                                    (lambda g=g: kproj(1, g)))
                            fills.setdefault(12, []).append(lambda: qproj(1, 0))
                    else:
                        # out-projection of the previous q-chunk: 8 units
                        # spread across the two pr iterations.
                        for i, slot in enumerate((6, 9, 12, 15)):
                            u = pr * 4 + i
                            st, nch = (qc - 1) * 4 + u // 2, u % 2
                            fills.setdefault(slot, []).append(
                                (lambda st=st, nch=nch: outproj_unit(st, nch)))
                        if pr == 0:
                            fills.setdefault(12, []).append(
                                (lambda qc=qc: qproj(qc, 1)))
                        elif qc < N_QC - 1:
                            fills.setdefault(12, []).append(
                                (lambda qc=qc: qproj(qc + 1, 0)))
                    pending_norm = attn_iter(qc, pr, fills)
            pending_norm(fast=True)

            # tail: out-projection of the last q-chunk
            for u in range(8):
                st, nch = (N_QC - 1) * 4 + u // 2, u % 2
                outproj_unit(st, nch)

    nc.compile()
    return nc


_NC_CACHE = None
last_in_maps = None


def kernel(x, Wq, bq, Wk, bk, Wv, bv, Wo, bo):
    global _NC_CACHE, last_in_maps
    x = np.asarray(x, dtype=np.float32)
    Wq, bq = np.asarray(Wq, np.float32), np.asarray(bq, np.float32)
    Wk, bk = np.asarray(Wk, np.float32), np.asarray(bk, np.float32)
    Wv, bv = np.asarray(Wv, np.float32), np.asarray(bv, np.float32)
    Wo, bo = np.asarray(Wo, np.float32), np.asarray(bo, np.float32)

    if _NC_CACHE is None:
        _NC_CACHE = build()
    nc = _NC_CACHE

    vce = np.zeros((128, N_KT, 128), np.float16)
    vce[:, :, 64] = 1.0
    vco = np.zeros((128, N_KT, 128), np.float16)
    vco[:, :, 0] = 1.0
    ones128 = np.ones((1, 128), np.float16)

    in_maps = []
    for c in range(8):
        b, f0 = c // 4, (c % 4) * E
        fs = slice(f0, f0 + E)
        in_maps.append(dict(
            xT=np.ascontiguousarray(x[b].T).astype(np.float16),
            wkq=np.ascontiguousarray(np.concatenate(
                [Wk[fs, :].T, Wq[fs, :].T], axis=1)).astype(np.float16),
            wvT=np.ascontiguousarray(Wv[fs, :].T).astype(np.float16),
            woT=np.ascontiguousarray(Wo[:, fs].T).astype(np.float16),
            bq2=np.ascontiguousarray(bq[fs].reshape(2, 128).T),
            bk2=np.ascontiguousarray(bk[fs].reshape(2, 128).T),
            bvb=np.ascontiguousarray(np.broadcast_to(bv[fs], (128, E))),
            vce=vce,
            vco=vco,
            ones128=ones128,
        ))

    last_in_maps = in_maps
    res = bass_utils.run_bass_kernel_spmd(nc, in_maps, core_ids=list(range(8)))

    out = np.zeros((B, S, D), np.float32)
    for c in range(8):
        out[c // 4] += res.results[c]["y"].astype(np.float32)
    out += bo
    return out



# revision 12
# speedup vs baseline: 1.0187x; 1.0187x over previous
"""Multi-head attention (B=2, S=2048, D=1024, H=16) on 8 TRN2 NeuronCores.

Sharding: core c handles batch b = c//4 and head group hg = c%4 (4 heads,
256 features f0 = hg*256). Each core computes Q/K/V projections for its
feature slice, attention for its 4 heads, and a partial output projection
y_partial = attnout @ Wo[:, f0:f0+256].T (emitted in fp16). Host sums the 4
partials per batch and adds bo.

Single fused software-pipelined loop, engineered so the tensor engine's
instruction stream is dense (TRN2 PE DVFS only reaches 2.4GHz after ~3us of
continuous execution):
 - K/V projections are JIT-streamed per k-tile-group inside q-chunk 0's
   attention loop; Q projections prefetched one (qc,pr) iteration ahead;
   out-projection of chunk qc deferred into chunk qc+1's loop. The PE
   therefore always has independent matmuls queued while softmax exp runs.
 - scores per head pair are issued back-to-back as K=64 matmuls on PE row
   groups 0:63 / 64:127 (tile_position auto-derived from base partition) so
   the two matmuls execute concurrently on the array.
 - softmax exp: scalar-engine Exp on [128, 2, 512] PSUM spans (two heads per
   instruction), with a fraction of k-tiles offloaded to DVE/Pool via a
   one-instruction fp16 Schraudolph exp (t = round(A*s + B) written as int16,
   bit-reinterpreted as fp16; constant-offset error is common-mode across k
   and cancels in the softmax normalization).
 - normalization: row sums ride along in the AV matmul (ones column in the
   augmented V); psav is drained early to SBUF (frees the PSUM bank), sums
   broadcast across partitions via two K=1 ones-matmuls into one PSUM tile,
   a single reciprocal_approx_fast over all 128 partitions, then two
   multiplies produce attnout.T in fp16.

All matmuls fp16 (PSUM accumulation fp32); elementwise fp32 on DVE/Pool.
"""
import numpy as np

import concourse.bass as bass
import concourse.mybir as mybir
import concourse.tile as tile
from concourse import bacc
from concourse import bass_utils

F32 = mybir.dt.float32
F16 = mybir.dt.float16
I16 = mybir.dt.int16
EXP = mybir.ActivationFunctionType.Exp
ADD = mybir.AluOpType.add
MULT = mybir.AluOpType.mult

B, S, D, H = 2, 2048, 1024, 16
HD = D // H          # 64
E = 256              # local features per core (4 heads)
QC = 512             # q-chunk size for the attention phase
N_QC = S // QC       # 4
N_KT = S // 128      # 16 k-tiles
KD = D // 128        # 8 contraction tiles for the projections

# Schraudolph fp16 exp: bits = round(x * 1024/ln2 + 15360 + C); the 0.125
# logit scale is folded into A. C=-44 minimizes max rel err (~3.1%); the
# constant-offset component cancels in the softmax normalization.
SCHR_A = 0.125 * 1024.0 / float(np.log(2.0))
SCHR_B = 15360.0 - 44.0
# k-tiles offloaded off the scalar engine per (qc, pr) iteration for qc >= 1
# (alternating DVE / Pool).
OFF_DVE = (3, 7, 11, 13)
OFF_POOL = ()
# dummy warmup matmuls (N=512): ~8 cold (3.4us, trips the HAM clock-gate to
# K=8/8) + a few warm ones to bridge until the first DMA-fed matmuls flow.
N_DUM = 12


def build():
    nc = bacc.Bacc("TRN2", target_bir_lowering=False, debug=False, num_devices=8)

    xT = nc.dram_tensor("xT", [D, S], F16, kind="ExternalInput").ap()
    # wkq = [wkT | wqT] concatenated host-side so one DMA per chunk loads both
    wkq = nc.dram_tensor("wkq", [D, 2 * E], F16, kind="ExternalInput").ap()
    wvT = nc.dram_tensor("wvT", [D, E], F16, kind="ExternalInput").ap()
    woT = nc.dram_tensor("woT", [E, D], F16, kind="ExternalInput").ap()
    bq2 = nc.dram_tensor("bq2", [128, 2], F32, kind="ExternalInput").ap()
    bk2 = nc.dram_tensor("bk2", [128, 2], F32, kind="ExternalInput").ap()
    bvb = nc.dram_tensor("bvb", [128, E], F32, kind="ExternalInput").ap()

    y = nc.dram_tensor("y", [S, D], F16, kind="ExternalOutput").ap()

    with tile.TileContext(nc) as tc:
        with (
            tc.tile_pool(name="pool", bufs=1) as pp,
            tc.tile_pool(name="pexp_pool", bufs=8) as pxp,
            tc.tile_pool(name="work", bufs=4) as wk,
            tc.tile_pool(name="rpool", bufs=2) as rp,
            tc.tile_pool(name="ypool", bufs=4) as yp,
            tc.tile_pool(name="ps_s", bufs=2, space="PSUM") as ps_s,
            tc.tile_pool(name="ps_av", bufs=2, space="PSUM") as ps_av,
            tc.tile_pool(name="ps_misc", bufs=2, space="PSUM") as ps_misc,
        ):
            # ---------------- persistent tiles ----------------
            woT_sb = pp.tile([128, 2, D], F16)
            bvb_sb = pp.tile([128, E], F32)
            # ones2: stationary for the fused sum-broadcast matmul:
            #   row 0  -> out partitions 64:128 (odd-head sums at col 0)
            #   row 64 -> out partitions 0:64  (even-head sums at col 64)
            ones2_sb = pp.tile([128, 128], F16)
            dum_sb = pp.tile([128, QC], F16)       # dummy warmup operand
            bq_sb = pp.tile([128, 2], F32)
            bk_sb = pp.tile([128, 2], F32)
            QT_sb = pp.tile([128, 2, S], F16)
            KT_sb = pp.tile([128, 2, S], F16)
            OT_sb = pp.tile([128, 2, S], F16)
            xT_sb = pp.tile([128, KD, S], F16)
            wkq_sb = pp.tile([128, KD, 2 * E], F16)
            wv_sb = pp.tile([128, KD, E], F16)
            # V_aug per pair (128 cols each so the matmul dst is a full
            # 128-partition AP):
            #   even head: [*, kt, 0:64]=V, col 64=1, cols 65:128=0
            #   odd head:  col 0=1, cols 1:64=0, [*, kt, 64:128]=V
            Ve_sb = [pp.tile([128, N_KT, 128], F16, name=f"ve{p}", tag=f"ve{p}")
                     for p in range(2)]
            Vo_sb = [pp.tile([128, N_KT, 128], F16, name=f"vo{p}", tag=f"vo{p}")
                     for p in range(2)]

            # ---------------- on-chip constant init + dummy PE warmup ------
            # The V_aug ones/zeros patterns and the broadcast stationary are
            # built with memsets (saves ~2.5MB of HBM traffic at warmup; the
            # input phase is DMA-bandwidth-bound). The dummy matmuls keep the
            # PE array busy from ~6us (right after the framework preamble) so
            # the HAM clock-gate reaches K=8/8 (~2.4GHz) at ~9.5us instead of
            # ~35us — the real early matmuls then run at full clock.
            nc.vector.memset(dum_sb[:], 0.0)
            nc.gpsimd.memset(ones2_sb[:], 0.0)
            nc.gpsimd.memset(ones2_sb[0:1, 64:128], 1.0)
            nc.gpsimd.memset(ones2_sb[64:65, 0:64], 1.0)
            for i in range(N_DUM):
                ps_d = ps_av.tile([128, QC], F32, tag="av")
                nc.tensor.matmul(ps_d[:], dum_sb[:, 0:128], dum_sb[:],
                                 start=True, stop=True)
            for pr in range(2):
                nc.vector.memset(Ve_sb[pr][:], 0.0)
                nc.vector.memset(Ve_sb[pr][:, :, 64:65], 1.0)
                nc.vector.memset(Vo_sb[pr][:], 0.0)
                nc.vector.memset(Vo_sb[pr][:, :, 0:1], 1.0)

            # ---------------- input DMAs ----------------
            # All transfers are 2-D [128, X] per-chunk DMAs (the fast DMA
            # path), spread over three engine queues (~0.6us issue cost per
            # dma_start per queue), ordered first-needed-first:
            # sync: all of xT; scalar: wkq + biases; gpsimd: wv + bvb + woT;
            # vector stays free for the V_aug memsets + early proj drains.
            for k in range(KD):
                nc.sync.dma_start(
                    xT_sb[:, k, 0:1024], xT[k * 128:(k + 1) * 128, 0:1024])
            for k in range(KD):
                nc.scalar.dma_start(
                    wkq_sb[:, k, :], wkq[k * 128:(k + 1) * 128, :])
            nc.scalar.dma_start(bk_sb[:], bk2)
            nc.scalar.dma_start(bq_sb[:], bq2)
            for k in range(KD):
                nc.gpsimd.dma_start(wv_sb[:, k, :], wvT[k * 128:(k + 1) * 128, :])
            nc.gpsimd.dma_start(bvb_sb[:], bvb)
            for p in range(2):
                nc.gpsimd.dma_start(woT_sb[:, p, :], woT[p * 128:(p + 1) * 128, :])
            for k in range(KD):
                nc.sync.dma_start(
                    xT_sb[:, k, 1024:2048], xT[k * 128:(k + 1) * 128, 1024:2048])

            # ---------------- emit helpers ----------------
            def proj_chain(w0, b_sb, out_sb, ch, g):
                """[128, 512] projection chunk: out_sb[:, ch, g*512:...] =
                W_ch.T @ xT[:, g-block] + b (drained on DVE). w0 selects the
                K (0) or Q (E) half of the fused wkq weights."""
                ssl = slice(g * 512, (g + 1) * 512)
                ps = ps_misc.tile([128, 512], F32, tag="misc")
                for k in range(KD):
                    nc.tensor.matmul(
                        ps[:],
                        wkq_sb[:, k, w0 + ch * 128:w0 + (ch + 1) * 128],
                        xT_sb[:, k, ssl],
                        start=(k == 0), stop=(k == KD - 1))
                nc.vector.tensor_scalar(
                    out_sb[:, ch, ssl], ps[:], b_sb[:, ch:ch + 1], None, ADD)

            def kproj(ch, g):
                proj_chain(0, bk_sb, KT_sb, ch, g)

            def qproj(qc, ch):
                proj_chain(E, bq_sb, QT_sb, ch, qc)

            def vproj(st):
                """V for s-tile st (all 4 heads), scattered+biased into the
                augmented V tiles on Pool."""
                ps = ps_misc.tile([128, 512], F32, tag="misc")
                for k in range(KD):
                    nc.tensor.matmul(
                        ps[:, 0:E],
                        xT_sb[:, k, st * 128:(st + 1) * 128],
                        wv_sb[:, k, :],
                        start=(k == 0), stop=(k == KD - 1))
                for h in range(4):
                    pr, odd = h // 2, h % 2
                    dst = (Vo_sb[pr][:, st, 64:128] if odd
                           else Ve_sb[pr][:, st, 0:64])
                    nc.vector.tensor_tensor(
                        dst, ps[:, h * 64:(h + 1) * 64],
                        bvb_sb[:, h * 64:(h + 1) * 64], ADD)

            def outproj_unit(st, nch, tail=False):
                """y[s-tile st, nch*512:...] = OT[:, :, ssl].T @ woT (both
                contraction chunks), drained to fp16, DMA'd out. In the tail
                (last q-chunk, after the loop) drains alternate DVE/ACT and
                DMAs rotate over four queues so the 8 units pipeline instead
                of serializing on one engine."""
                ssl = slice(st * 128, (st + 1) * 128)
                psy = ps_misc.tile([128, 512], F32, tag="misc")
                for cc in range(2):
                    nc.tensor.matmul(
                        psy[:], OT_sb[:, cc, ssl],
                        woT_sb[:, cc, nch * 512:(nch + 1) * 512],
                        start=(cc == 0), stop=(cc == 1))
                y_sb = yp.tile([128, 512], F16, tag="y")
                u = st * 2 + nch
                if tail and u % 2 == 1:
                    nc.scalar.copy(y_sb[:], psy[:])
                else:
                    nc.vector.tensor_copy(y_sb[:], psy[:])
                if tail:
                    deng = (nc.sync, nc.gpsimd, nc.scalar)[u % 3]
                else:
                    deng = nc.sync if nch == 0 else nc.gpsimd
                deng.dma_start(y[ssl, nch * 512:(nch + 1) * 512], y_sb[:])

            def attn_iter(qc, pr, fills):
                """One (q-chunk, head-pair) attention iteration. `fills` maps
                kt -> list of emit callables sprinkled into the loop to keep
                the PE stream dense. AV matmuls run AV_LAG k-tiles behind the
                score matmuls so the PE never waits on the exp latency.
                Returns a callable that emits the PE/DVE/Pool back half of the
                normalization (scheduled as a fill in the next iteration)."""
                qsl = slice(qc * QC, (qc + 1) * QC)
                av_e = ps_av.tile([128, QC], F32, tag="av")
                av_o = ps_av.tile([128, QC], F32, tag="av")
                pexps = {}
                AV_LAG = 3

                def emit_av(kt):
                    pexp = pexps.pop(kt)
                    nc.tensor.matmul(av_e[:], Ve_sb[pr][:, kt, :],
                                     pexp[:, 0, :],
                                     start=(kt == 0), stop=(kt == N_KT - 1))
                    nc.tensor.matmul(av_o[:], Vo_sb[pr][:, kt, :],
                                     pexp[:, 1, :],
                                     start=(kt == 0), stop=(kt == N_KT - 1))

                for kt in range(N_KT):
                    for f in fills.get(kt, ()):
                        f()
                    ksl = slice(kt * 128, (kt + 1) * 128)
                    # scores for the head pair: two K=64 matmuls on PE row
                    # groups 0:63 / 64:127, issued back-to-back so they run
                    # concurrently on the array.
                    ps = ps_s.tile([128, 2, QC], F32, tag="s")
                    nc.tensor.matmul(ps[:, 0, :], KT_sb[0:64, pr, ksl],
                                     QT_sb[0:64, pr, qsl])
                    nc.tensor.matmul(ps[:, 1, :], KT_sb[64:128, pr, ksl],
                                     QT_sb[64:128, pr, qsl])
                    pexp = pxp.tile([128, 2, QC], F16, tag="pexp")
                    pexps[kt] = pexp
                    if qc > 0 and kt in OFF_DVE:
                        nc.vector.tensor_scalar(
                            pexp[:].bitcast(I16), ps[:], SCHR_A, SCHR_B,
                            MULT, ADD)
                    else:
                        nc.scalar.activation(pexp[:], ps[:], EXP, scale=0.125)
                    if kt >= AV_LAG:
                        emit_av(kt - AV_LAG)
                for kt in range(N_KT - AV_LAG, N_KT):
                    emit_av(kt)
                # normalization, front half (DVE): sums rows to SBUF fp16
                # first (unblocks the broadcast matmuls), then drain psav to
                # SBUF (frees the PSUM banks for the next iteration).
                avsb_e = wk.tile([128, QC], F32, tag="avsb")
                avsb_o = wk.tile([128, QC], F32, tag="avsb")
                sums16 = wk.tile([128, QC], F16, tag="sums16")
                # av_o partitions 1:64 are exact zeros (V_aug odd cols 1:64
                # are zero), so this one copy fills sums16[0:64] with
                # [sums_o, 0, 0, ...] — making the K=65 broadcast matmul
                # below safe (no uninitialized SBUF enters the array).
                nc.vector.tensor_copy(sums16[0:64, :], av_o[0:64, :])
                nc.vector.tensor_copy(sums16[64:65, :], av_e[64:65, :])
                nc.vector.tensor_copy(avsb_e[:], av_e[:])
                nc.vector.tensor_copy(avsb_o[:], av_o[:])

                def finish_norm(fast=False):
                    # broadcast both raw sums (partition 64 = even head,
                    # partition 0 = odd head) with ONE K=65 ones-matmul into
                    # one PSUM tile, one approx reciprocal over all 128
                    # partitions, then scale on Pool (all-SBUF operands).
                    psbc = ps_misc.tile([128, 512], F32, tag="misc")
                    nc.tensor.matmul(psbc[:], ones2_sb[0:65, :],
                                     sums16[0:65, :])
                    rec = rp.tile([128, QC], F32, tag="rec")
                    nc.vector.reciprocal_approx_fast(rec[:], psbc[:])
                    eng = nc.vector if (fast or pr == 1) else nc.gpsimd
                    eng.tensor_tensor(
                        OT_sb[0:64, pr, qsl], avsb_e[0:64, :], rec[0:64, :],
                        MULT)
                    eng.tensor_tensor(
                        OT_sb[64:128, pr, qsl], avsb_o[64:128, :],
                        rec[64:128, :], MULT)

                return finish_norm

            # ---------------- fused main loop ----------------
            # Warmup: K/Q first (their weights lead the scalar queue; scores
            # can then start early, warming up the scalar engine), V after
            # (its weights stream in on the gpsimd queue meanwhile).
            kproj(0, 0)
            qproj(0, 0)
            vproj(0)
            vproj(1)
            kproj(0, 1)
            vproj(2)
            vproj(3)
            vproj(4)
            vproj(5)

            pending_norm = None
            for qc in range(N_QC):
                for pr in range(2):
                    fills = {}
                    if pending_norm is not None:
                        fills.setdefault(1, []).append(pending_norm)
                    if qc == 0:
                        # JIT K/V projections. K chunk `pr` group g must
                        # precede scores kt=4g; V tile st must precede
                        # AV kt=st (prefetch distance 2 + AV lag).
                        if pr == 0:
                            for g in range(2, 4):
                                fills.setdefault(4 * (g - 1) + 2, []).append(
                                    (lambda g=g: kproj(0, g)))
                            for st in range(6, N_KT):
                                fills.setdefault(st - 6, []).append(
                                    (lambda st=st: vproj(st)))
                            fills.setdefault(12, []).append(lambda: qproj(0, 1))
                            fills.setdefault(13, []).append(lambda: kproj(1, 0))
                        else:
                            for g in range(1, 4):
                                fills.setdefault(4 * (g - 1) + 2, []).append(
                                    (lambda g=g: kproj(1, g)))
                            fills.setdefault(12, []).append(lambda: qproj(1, 0))
                    else:
                        # out-projection of the previous q-chunk: 8 units
                        # spread across the two pr iterations.
                        for i, slot in enumerate((6, 9, 12, 15)):
                            u = pr * 4 + i
                            st, nch = (qc - 1) * 4 + u // 2, u % 2
                            fills.setdefault(slot, []).append(
                                (lambda st=st, nch=nch: outproj_unit(st, nch)))
                        if pr == 0:
                            fills.setdefault(12, []).append(
                                (lambda qc=qc: qproj(qc, 1)))
                        elif qc < N_QC - 1:
                            fills.setdefault(12, []).append(
                                (lambda qc=qc: qproj(qc + 1, 0)))
                    pending_norm = attn_iter(qc, pr, fills)
            pending_norm(fast=True)

            # tail: out-projection of the last q-chunk
            for u in range(8):
                st, nch = (N_QC - 1) * 4 + u // 2, u % 2
                outproj_unit(st, nch, tail=True)

    nc.compile()
    return nc


_NC_CACHE = None
last_in_maps = None


def kernel(x, Wq, bq, Wk, bk, Wv, bv, Wo, bo):
    global _NC_CACHE, last_in_maps
    x = np.asarray(x, dtype=np.float32)
    Wq, bq = np.asarray(Wq, np.float32), np.asarray(bq, np.float32)
    Wk, bk = np.asarray(Wk, np.float32), np.asarray(bk, np.float32)
    Wv, bv = np.asarray(Wv, np.float32), np.asarray(bv, np.float32)
    Wo, bo = np.asarray(Wo, np.float32), np.asarray(bo, np.float32)

    if _NC_CACHE is None:
        _NC_CACHE = build()
    nc = _NC_CACHE

    in_maps = []
    for c in range(8):
        b, f0 = c // 4, (c % 4) * E
        fs = slice(f0, f0 + E)
        in_maps.append(dict(
            xT=np.ascontiguousarray(x[b].T).astype(np.float16),
            wkq=np.ascontiguousarray(np.concatenate(
                [Wk[fs, :].T, Wq[fs, :].T], axis=1)).astype(np.float16),
            wvT=np.ascontiguousarray(Wv[fs, :].T).astype(np.float16),
            woT=np.ascontiguousarray(Wo[:, fs].T).astype(np.float16),
            bq2=np.ascontiguousarray(bq[fs].reshape(2, 128).T),
            bk2=np.ascontiguousarray(bk[fs].reshape(2, 128).T),
            bvb=np.ascontiguousarray(np.broadcast_to(bv[fs], (128, E))),
        ))

    last_in_maps = in_maps
    res = bass_utils.run_bass_kernel_spmd(nc, in_maps, core_ids=list(range(8)))

    out = np.zeros((B, S, D), np.float32)
    for c in range(8):
        out[c // 4] += res.results[c]["y"].astype(np.float32)
    out += bo
    return out



# revision 20
# speedup vs baseline: 1.0375x; 1.0185x over previous
"""Multi-head attention (B=2, S=2048, D=1024, H=16) on 8 TRN2 NeuronCores.

Sharding: core c handles batch b = c//4 and head group hg = c%4 (4 heads,
256 features f0 = hg*256). Each core computes Q/K/V projections for its
feature slice, attention for its 4 heads, and a partial output projection
y_partial = attnout @ Wo[:, f0:f0+256].T (emitted in fp16). Host sums the 4
partials per batch and adds bo.

Single fused software-pipelined loop, engineered so the tensor engine's
instruction stream is dense (TRN2 PE DVFS only reaches 2.4GHz after ~3us of
continuous execution):
 - K/V projections are JIT-streamed per k-tile-group inside q-chunk 0's
   attention loop; Q projections prefetched one (qc,pr) iteration ahead;
   out-projection of chunk qc deferred into chunk qc+1's loop. The PE
   therefore always has independent matmuls queued while softmax exp runs.
 - scores per head pair are issued back-to-back as K=64 matmuls on PE row
   groups 0:63 / 64:127 (tile_position auto-derived from base partition) so
   the two matmuls execute concurrently on the array.
 - softmax exp: scalar-engine Exp on [128, 2, 512] PSUM spans (two heads per
   instruction), with a fraction of k-tiles offloaded to DVE/Pool via a
   one-instruction fp16 Schraudolph exp (t = round(A*s + B) written as int16,
   bit-reinterpreted as fp16; constant-offset error is common-mode across k
   and cancels in the softmax normalization).
 - normalization: row sums ride along in the AV matmul (ones column in the
   augmented V); psav is drained early to SBUF (frees the PSUM bank), sums
   broadcast across partitions via two K=1 ones-matmuls into one PSUM tile,
   a single reciprocal_approx_fast over all 128 partitions, then two
   multiplies produce attnout.T in fp16.

All matmuls fp16 (PSUM accumulation fp32); elementwise fp32 on DVE/Pool.
"""
import numpy as np

import concourse.bass as bass
import concourse.mybir as mybir
import concourse.tile as tile
from concourse import bacc
from concourse import bass_utils

F32 = mybir.dt.float32
F16 = mybir.dt.float16
I16 = mybir.dt.int16
EXP = mybir.ActivationFunctionType.Exp
ADD = mybir.AluOpType.add
MULT = mybir.AluOpType.mult

B, S, D, H = 2, 2048, 1024, 16
HD = D // H          # 64
E = 256              # local features per core (4 heads)
QC = 512             # q-chunk size for the attention phase
N_QC = S // QC       # 4
N_KT = S // 128      # 16 k-tiles
KD = D // 128        # 8 contraction tiles for the projections

# Schraudolph fp16 exp: bits = round(x * 1024/ln2 + 15360 + C); the 0.125
# logit scale is folded into A. C=-44 minimizes max rel err (~3.1%); the
# constant-offset component cancels in the softmax normalization.
SCHR_A = 0.125 * 1024.0 / float(np.log(2.0))
SCHR_B = 15360.0 - 44.0
# k-tiles offloaded off the scalar engine per (qc, pr) iteration for qc >= 1
# (alternating DVE / Pool).
OFF_DVE = (3, 7, 11, 13)
OFF_POOL = ()
# dummy warmup matmuls (N=512): bridge the gap between the framework preamble
# (~6us) and the first DMA-fed matmuls (~8us) so the PE activity window that
# drives the HAM clock-gate starts as early as possible.
N_DUM = 4


def build():
    nc = bacc.Bacc("TRN2", target_bir_lowering=False, debug=False, num_devices=8)

    xT = nc.dram_tensor("xT", [D, S], F16, kind="ExternalInput").ap()
    # wkq = [wkT | wqT] concatenated host-side so one DMA per chunk loads both
    wkq = nc.dram_tensor("wkq", [D, 2 * E], F16, kind="ExternalInput").ap()
    wvT = nc.dram_tensor("wvT", [D, E], F16, kind="ExternalInput").ap()
    woT = nc.dram_tensor("woT", [E, D], F16, kind="ExternalInput").ap()
    bq2 = nc.dram_tensor("bq2", [128, 2], F32, kind="ExternalInput").ap()
    bk2 = nc.dram_tensor("bk2", [128, 2], F32, kind="ExternalInput").ap()
    bvb = nc.dram_tensor("bvb", [128, E], F32, kind="ExternalInput").ap()

    y = nc.dram_tensor("y", [S, D], F16, kind="ExternalOutput").ap()

    with tile.TileContext(nc) as tc:
        with (
            tc.tile_pool(name="pool", bufs=1) as pp,
            tc.tile_pool(name="pexp_pool", bufs=8) as pxp,
            tc.tile_pool(name="work", bufs=4) as wk,
            tc.tile_pool(name="rpool", bufs=2) as rp,
            tc.tile_pool(name="ypool", bufs=4) as yp,
            tc.tile_pool(name="ps_s", bufs=2, space="PSUM") as ps_s,
            tc.tile_pool(name="ps_av", bufs=2, space="PSUM") as ps_av,
            tc.tile_pool(name="ps_misc", bufs=2, space="PSUM") as ps_misc,
        ):
            # ---------------- persistent tiles ----------------
            woT_sb = pp.tile([128, 2, D], F16)
            bvb_sb = pp.tile([128, E], F32)
            # ones2: stationary for the fused sum-broadcast matmul:
            #   row 0  -> out partitions 64:128 (odd-head sums at col 0)
            #   row 64 -> out partitions 0:64  (even-head sums at col 64)
            ones2_sb = pp.tile([128, 128], F16)
            dum_sb = pp.tile([128, QC], F16)       # dummy warmup operand
            bq_sb = pp.tile([128, 2], F32)
            bk_sb = pp.tile([128, 2], F32)
            QT_sb = pp.tile([128, 2, S], F16)
            KT_sb = pp.tile([128, 2, S], F16)
            OT_sb = pp.tile([128, 2, S], F16)
            xT_sb = pp.tile([128, KD, S], F16)
            wkq_sb = pp.tile([128, KD, 2 * E], F16)
            wv_sb = pp.tile([128, KD, E], F16)
            # V_aug per pair (128 cols each so the matmul dst is a full
            # 128-partition AP):
            #   even head: [*, kt, 0:64]=V, col 64=1, cols 65:128=0
            #   odd head:  col 0=1, cols 1:64=0, [*, kt, 64:128]=V
            Ve_sb = [pp.tile([128, N_KT, 128], F16, name=f"ve{p}", tag=f"ve{p}")
                     for p in range(2)]
            Vo_sb = [pp.tile([128, N_KT, 128], F16, name=f"vo{p}", tag=f"vo{p}")
                     for p in range(2)]

            # ---------------- on-chip constant init + dummy PE warmup ------
            # The V_aug ones/zeros patterns and the broadcast stationary are
            # built with memsets (saves ~2.5MB of HBM traffic at warmup; the
            # input phase is DMA-bandwidth-bound). The dummy matmuls keep the
            # PE array busy from ~6us (right after the framework preamble) so
            # the HAM clock-gate reaches K=8/8 (~2.4GHz) at ~9.5us instead of
            # ~35us — the real early matmuls then run at full clock.
            nc.vector.memset(dum_sb[:], 0.0)
            nc.gpsimd.memset(ones2_sb[:], 0.0)
            nc.gpsimd.memset(ones2_sb[0:1, 64:128], 1.0)
            nc.gpsimd.memset(ones2_sb[64:65, 0:64], 1.0)
            for i in range(N_DUM):
                ps_d = ps_av.tile([128, QC], F32, tag="av")
                nc.tensor.matmul(ps_d[:], dum_sb[:, 0:128], dum_sb[:],
                                 start=True, stop=True)
            for pr in range(2):
                nc.vector.memset(Ve_sb[pr][:], 0.0)
                nc.vector.memset(Ve_sb[pr][:, :, 64:65], 1.0)
                nc.vector.memset(Vo_sb[pr][:], 0.0)
                nc.vector.memset(Vo_sb[pr][:, :, 0:1], 1.0)

            # ---------------- input DMAs ----------------
            # Per-queue DMA transfers serialize at ~1.3-3us apiece, so the
            # ~6MB of input is BALANCED across the three DMA-capable queues
            # (~2MB each), ordered first-needed-first:
            # sync: xT s0:1024; scalar: biases + wkq + xT-hi k0:4;
            # gpsimd: wv + bvb + xT-hi k4:8 + woT.
            nc.scalar.dma_start(bk_sb[:], bk2)
            nc.scalar.dma_start(bq_sb[:], bq2)
            for k in range(KD):
                nc.sync.dma_start(
                    xT_sb[:, k, 0:1024], xT[k * 128:(k + 1) * 128, 0:1024])
            for k in range(KD):
                nc.scalar.dma_start(
                    wkq_sb[:, k, :], wkq[k * 128:(k + 1) * 128, :])
            for k in range(KD):
                nc.gpsimd.dma_start(wv_sb[:, k, :], wvT[k * 128:(k + 1) * 128, :])
            nc.gpsimd.dma_start(bvb_sb[:], bvb)
            for k in range(KD):
                eng = nc.scalar if k < 4 else nc.gpsimd
                eng.dma_start(
                    xT_sb[:, k, 1024:2048], xT[k * 128:(k + 1) * 128, 1024:2048])
            for p in range(2):
                nc.gpsimd.dma_start(woT_sb[:, p, :], woT[p * 128:(p + 1) * 128, :])

            # ---------------- emit helpers ----------------
            def proj_chain(w0, b_sb, out_sb, ch, g):
                """[128, 512] projection chunk: out_sb[:, ch, g*512:...] =
                W_ch.T @ xT[:, g-block] + b (drained on DVE). w0 selects the
                K (0) or Q (E) half of the fused wkq weights."""
                ssl = slice(g * 512, (g + 1) * 512)
                ps = ps_misc.tile([128, 512], F32, tag="misc")
                for k in range(KD):
                    nc.tensor.matmul(
                        ps[:],
                        wkq_sb[:, k, w0 + ch * 128:w0 + (ch + 1) * 128],
                        xT_sb[:, k, ssl],
                        start=(k == 0), stop=(k == KD - 1))
                nc.vector.tensor_scalar(
                    out_sb[:, ch, ssl], ps[:], b_sb[:, ch:ch + 1], None, ADD)

            def kproj(ch, g):
                proj_chain(0, bk_sb, KT_sb, ch, g)

            def qproj(qc, ch):
                proj_chain(E, bq_sb, QT_sb, ch, qc)

            def vproj(st):
                """V for s-tile st (all 4 heads), scattered+biased into the
                augmented V tiles on Pool."""
                ps = ps_misc.tile([128, 512], F32, tag="misc")
                for k in range(KD):
                    nc.tensor.matmul(
                        ps[:, 0:E],
                        xT_sb[:, k, st * 128:(st + 1) * 128],
                        wv_sb[:, k, :],
                        start=(k == 0), stop=(k == KD - 1))
                for h in range(4):
                    pr, odd = h // 2, h % 2
                    dst = (Vo_sb[pr][:, st, 64:128] if odd
                           else Ve_sb[pr][:, st, 0:64])
                    nc.vector.tensor_tensor(
                        dst, ps[:, h * 64:(h + 1) * 64],
                        bvb_sb[:, h * 64:(h + 1) * 64], ADD)

            def outproj_unit(st, nch, tail=False):
                """y[s-tile st, nch*512:...] = OT[:, :, ssl].T @ woT (both
                contraction chunks), drained to fp16, DMA'd out. In the tail
                (last q-chunk, after the loop) drains alternate DVE/ACT and
                DMAs rotate over four queues so the 8 units pipeline instead
                of serializing on one engine."""
                ssl = slice(st * 128, (st + 1) * 128)
                psy = ps_misc.tile([128, 512], F32, tag="misc")
                for cc in range(2):
                    nc.tensor.matmul(
                        psy[:], OT_sb[:, cc, ssl],
                        woT_sb[:, cc, nch * 512:(nch + 1) * 512],
                        start=(cc == 0), stop=(cc == 1))
                y_sb = yp.tile([128, 512], F16, tag="y")
                u = st * 2 + nch
                if tail and u % 2 == 1:
                    nc.scalar.copy(y_sb[:], psy[:])
                else:
                    nc.vector.tensor_copy(y_sb[:], psy[:])
                if tail:
                    deng = (nc.sync, nc.gpsimd, nc.scalar)[u % 3]
                else:
                    deng = nc.sync if nch == 0 else nc.gpsimd
                deng.dma_start(y[ssl, nch * 512:(nch + 1) * 512], y_sb[:])

            def attn_iter(qc, pr, fills, last=False):
                """One (q-chunk, head-pair) attention iteration. `fills` maps
                kt -> list of emit callables sprinkled into the loop to keep
                the PE stream dense. AV matmuls run AV_LAG k-tiles behind the
                score matmuls so the PE never waits on the exp latency.
                Returns a callable that emits the PE/DVE/Pool back half of the
                normalization (scheduled as a fill in the next iteration).
                last=True shortens the exposed end-of-kernel critical path:
                the trailing exps move to DVE (the scalar queue lags), the
                sums copies split across ACT/DVE, and the normalization
                multiplies read the AV psum directly (no need to free the
                banks for a next iteration)."""
                qsl = slice(qc * QC, (qc + 1) * QC)
                av_e = ps_av.tile([128, QC], F32, tag="av")
                av_o = ps_av.tile([128, QC], F32, tag="av")
                pexps = {}
                AV_LAG = 3

                def emit_av(kt):
                    pexp = pexps.pop(kt)
                    nc.tensor.matmul(av_e[:], Ve_sb[pr][:, kt, :],
                                     pexp[:, 0, :],
                                     start=(kt == 0), stop=(kt == N_KT - 1))
                    nc.tensor.matmul(av_o[:], Vo_sb[pr][:, kt, :],
                                     pexp[:, 1, :],
                                     start=(kt == 0), stop=(kt == N_KT - 1))

                for kt in range(N_KT):
                    for f in fills.get(kt, ()):
                        f()
                    ksl = slice(kt * 128, (kt + 1) * 128)
                    # scores for the head pair: two K=64 matmuls on PE row
                    # groups 0:63 / 64:127, issued back-to-back so they run
                    # concurrently on the array.
                    ps = ps_s.tile([128, 2, QC], F32, tag="s")
                    nc.tensor.matmul(ps[:, 0, :], KT_sb[0:64, pr, ksl],
                                     QT_sb[0:64, pr, qsl])
                    nc.tensor.matmul(ps[:, 1, :], KT_sb[64:128, pr, ksl],
                                     QT_sb[64:128, pr, qsl])
                    pexp = pxp.tile([128, 2, QC], F16, tag="pexp")
                    pexps[kt] = pexp
                    offs = OFF_DVE + ((14, 15) if last else ())
                    if qc > 0 and kt in offs:
                        nc.vector.tensor_scalar(
                            pexp[:].bitcast(I16), ps[:], SCHR_A, SCHR_B,
                            MULT, ADD)
                    else:
                        nc.scalar.activation(pexp[:], ps[:], EXP, scale=0.125)
                    if kt >= AV_LAG:
                        emit_av(kt - AV_LAG)
                for kt in range(N_KT - AV_LAG, N_KT):
                    emit_av(kt)
                # normalization, front half (DVE): sums rows to SBUF fp16
                # first (unblocks the broadcast matmuls), then drain psav to
                # SBUF (frees the PSUM banks for the next iteration).
                sums16 = wk.tile([128, QC], F16, tag="sums16")
                # av_o partitions 1:64 are exact zeros (V_aug odd cols 1:64
                # are zero), so this one copy fills sums16[0:64] with
                # [sums_o, 0, 0, ...] — making the K=65 broadcast matmul
                # below safe (no uninitialized SBUF enters the array).
                if last:
                    nc.scalar.copy(sums16[0:64, :], av_o[0:64, :])
                    nc.vector.tensor_copy(sums16[64:65, :], av_e[64:65, :])
                    avsb_e = avsb_o = None
                else:
                    nc.vector.tensor_copy(sums16[0:64, :], av_o[0:64, :])
                    nc.vector.tensor_copy(sums16[64:65, :], av_e[64:65, :])
                    avsb_e = wk.tile([128, QC], F32, tag="avsb")
                    avsb_o = wk.tile([128, QC], F32, tag="avsb")
                    nc.vector.tensor_copy(avsb_e[:], av_e[:])
                    nc.vector.tensor_copy(avsb_o[:], av_o[:])

                def finish_norm(fast=False):
                    # broadcast both raw sums (partition 64 = even head,
                    # partition 0 = odd head) with ONE K=65 ones-matmul into
                    # one PSUM tile, one approx reciprocal over all 128
                    # partitions, then scale on Pool (all-SBUF operands).
                    psbc = ps_misc.tile([128, 512], F32, tag="misc")
                    nc.tensor.matmul(psbc[:], ones2_sb[0:65, :],
                                     sums16[0:65, :])
                    rec = rp.tile([128, QC], F32, tag="rec")
                    nc.vector.reciprocal_approx_fast(rec[:], psbc[:])
                    if last:
                        # read the AV psum directly — the banks are not
                        # needed again, and it removes two serial DVE copies
                        # from the end-of-kernel critical path.
                        nc.vector.tensor_tensor(
                            OT_sb[0:64, pr, qsl], av_e[0:64, :], rec[0:64, :],
                            MULT)
                        nc.vector.tensor_tensor(
                            OT_sb[64:128, pr, qsl], av_o[64:128, :],
                            rec[64:128, :], MULT)
                        return
                    eng = nc.vector if (fast or pr == 1) else nc.gpsimd
                    eng.tensor_tensor(
                        OT_sb[0:64, pr, qsl], avsb_e[0:64, :], rec[0:64, :],
                        MULT)
                    eng.tensor_tensor(
                        OT_sb[64:128, pr, qsl], avsb_o[64:128, :],
                        rec[64:128, :], MULT)

                return finish_norm

            # ---------------- fused main loop ----------------
            # Warmup phase A, k-outer: the input DMAs deliver one k-chunk of
            # xT/wkq/wv every ~1.3us per queue; iterating the CONTRACTION
            # index outermost lets every arriving chunk unlock one matmul in
            # each of 8 concurrent accumulation chains (kproj g0/g1, qproj
            # g0, vproj st0-4), keeping the PE dense from the very first
            # chunk. The chains borrow the attention pools' PSUM banks
            # (idle until ~18us).
            wu_k0 = ps_misc.tile([128, 512], F32, tag="misc")
            wu_q0 = ps_misc.tile([128, 512], F32, tag="misc")
            wu_s = [ps_s.tile([128, 2, QC], F32, tag="s", name=f"wu_s{i}")
                    for i in range(2)]
            wu_a = [ps_av.tile([128, QC], F32, tag="av", name=f"wu_a{i}")
                    for i in range(2)]
            vp_ps = [wu_s[0][:, 1, 0:E], wu_s[1][:, 0, 0:E],
                     wu_s[1][:, 1, 0:E], wu_a[0][:, 0:E], wu_a[1][:, 0:E]]
            for k in range(KD):
                st_, sp_ = (k == 0), (k == KD - 1)
                for st in range(5):
                    nc.tensor.matmul(
                        vp_ps[st], xT_sb[:, k, st * 128:(st + 1) * 128],
                        wv_sb[:, k, :], start=st_, stop=sp_)
                nc.tensor.matmul(wu_k0[:], wkq_sb[:, k, 0:128],
                                 xT_sb[:, k, 0:512], start=st_, stop=sp_)
                nc.tensor.matmul(wu_q0[:], wkq_sb[:, k, E:E + 128],
                                 xT_sb[:, k, 0:512], start=st_, stop=sp_)
                nc.tensor.matmul(wu_s[0][:, 0, :], wkq_sb[:, k, 0:128],
                                 xT_sb[:, k, 512:1024], start=st_, stop=sp_)
            # phase B: drains (KT g0 + QT first — scores kt0 needs them)
            nc.vector.tensor_scalar(
                KT_sb[:, 0, 0:512], wu_k0[:], bk_sb[:, 0:1], None, ADD)
            nc.vector.tensor_scalar(
                QT_sb[:, 0, 0:512], wu_q0[:], bq_sb[:, 0:1], None, ADD)
            nc.vector.tensor_scalar(
                KT_sb[:, 0, 512:1024], wu_s[0][:, 0, :], bk_sb[:, 0:1],
                None, ADD)
            for st in range(5):
                for h in range(4):
                    pr_, odd = h // 2, h % 2
                    dst = (Vo_sb[pr_][:, st, 64:128] if odd
                           else Ve_sb[pr_][:, st, 0:64])
                    nc.vector.tensor_tensor(
                        dst, vp_ps[st][:, h * 64:(h + 1) * 64],
                        bvb_sb[:, h * 64:(h + 1) * 64], ADD)
            vproj(5)

            pending_norm = None
            for qc in range(N_QC):
                for pr in range(2):
                    fills = {}
                    if pending_norm is not None:
                        fills.setdefault(1, []).append(pending_norm)
                    if qc == 0:
                        # JIT K/V projections. K chunk `pr` group g must
                        # precede scores kt=4g; V tile st must precede
                        # AV kt=st (prefetch distance 2 + AV lag).
                        if pr == 0:
                            for g in range(2, 4):
                                fills.setdefault(4 * (g - 1) + 2, []).append(
                                    (lambda g=g: kproj(0, g)))
                            for st in range(6, N_KT):
                                fills.setdefault(st - 6, []).append(
                                    (lambda st=st: vproj(st)))
                            fills.setdefault(12, []).append(lambda: qproj(0, 1))
                            fills.setdefault(13, []).append(lambda: kproj(1, 0))
                        else:
                            for g in range(1, 4):
                                fills.setdefault(4 * (g - 1) + 2, []).append(
                                    (lambda g=g: kproj(1, g)))
                            fills.setdefault(12, []).append(lambda: qproj(1, 0))
                    else:
                        # out-projection of the previous q-chunk: 8 units
                        # spread across the two pr iterations.
                        for i, slot in enumerate((6, 9, 12, 15)):
                            u = pr * 4 + i
                            st, nch = (qc - 1) * 4 + u // 2, u % 2
                            fills.setdefault(slot, []).append(
                                (lambda st=st, nch=nch: outproj_unit(st, nch)))
                        if pr == 0:
                            fills.setdefault(12, []).append(
                                (lambda qc=qc: qproj(qc, 1)))
                        elif qc < N_QC - 1:
                            fills.setdefault(12, []).append(
                                (lambda qc=qc: qproj(qc + 1, 0)))
                    pending_norm = attn_iter(
                        qc, pr, fills,
                        last=(qc == N_QC - 1 and pr == 1))
            pending_norm(fast=True)

            # tail: out-projection of the last q-chunk
            for u in range(8):
                st, nch = (N_QC - 1) * 4 + u // 2, u % 2
                outproj_unit(st, nch, tail=True)

    nc.compile()
    return nc


_NC_CACHE = None
last_in_maps = None


def kernel(x, Wq, bq, Wk, bk, Wv, bv, Wo, bo):
    global _NC_CACHE, last_in_maps
    x = np.asarray(x, dtype=np.float32)
    Wq, bq = np.asarray(Wq, np.float32), np.asarray(bq, np.float32)
    Wk, bk = np.asarray(Wk, np.float32), np.asarray(bk, np.float32)
    Wv, bv = np.asarray(Wv, np.float32), np.asarray(bv, np.float32)
    Wo, bo = np.asarray(Wo, np.float32), np.asarray(bo, np.float32)

    if _NC_CACHE is None:
        _NC_CACHE = build()
    nc = _NC_CACHE

    in_maps = []
    for c in range(8):
        b, f0 = c // 4, (c % 4) * E
        fs = slice(f0, f0 + E)
        in_maps.append(dict(
            xT=np.ascontiguousarray(x[b].T).astype(np.float16),
            wkq=np.ascontiguousarray(np.concatenate(
                [Wk[fs, :].T, Wq[fs, :].T], axis=1)).astype(np.float16),
            wvT=np.ascontiguousarray(Wv[fs, :].T).astype(np.float16),
            woT=np.ascontiguousarray(Wo[:, fs].T).astype(np.float16),
            bq2=np.ascontiguousarray(bq[fs].reshape(2, 128).T),
            bk2=np.ascontiguousarray(bk[fs].reshape(2, 128).T),
            bvb=np.ascontiguousarray(np.broadcast_to(bv[fs], (128, E))),
        ))

    last_in_maps = in_maps
    res = bass_utils.run_bass_kernel_spmd(nc, in_maps, core_ids=list(range(8)))

    out = np.zeros((B, S, D), np.float32)
    for c in range(8):
        out[c // 4] += res.results[c]["y"].astype(np.float32)
    out += bo
    return out



# revision 25
# speedup vs baseline: 1.0503x; 1.0123x over previous
"""Multi-head attention (B=2, S=2048, D=1024, H=16) on 8 TRN2 NeuronCores.

Sharding: core c handles batch b = c//4 and head group hg = c%4 (4 heads,
256 features f0 = hg*256). Each core computes Q/K/V projections for its
feature slice, attention for its 4 heads, and a partial output projection
y_partial = attnout @ Wo[:, f0:f0+256].T (emitted in fp16). Host sums the 4
partials per batch and adds bo.

Single fused software-pipelined loop, engineered so the tensor engine's
instruction stream is dense (TRN2 PE DVFS only reaches 2.4GHz after ~3us of
continuous execution):
 - K/V projections are JIT-streamed per k-tile-group inside q-chunk 0's
   attention loop; Q projections prefetched one (qc,pr) iteration ahead;
   out-projection of chunk qc deferred into chunk qc+1's loop. The PE
   therefore always has independent matmuls queued while softmax exp runs.
 - scores per head pair are issued back-to-back as K=64 matmuls on PE row
   groups 0:63 / 64:127 (tile_position auto-derived from base partition) so
   the two matmuls execute concurrently on the array.
 - softmax exp: scalar-engine Exp on [128, 2, 512] PSUM spans (two heads per
   instruction), with a fraction of k-tiles offloaded to DVE/Pool via a
   one-instruction fp16 Schraudolph exp (t = round(A*s + B) written as int16,
   bit-reinterpreted as fp16; constant-offset error is common-mode across k
   and cancels in the softmax normalization).
 - normalization: row sums ride along in the AV matmul (ones column in the
   augmented V); psav is drained early to SBUF (frees the PSUM bank), sums
   broadcast across partitions via two K=1 ones-matmuls into one PSUM tile,
   a single reciprocal_approx_fast over all 128 partitions, then two
   multiplies produce attnout.T in fp16.

All matmuls fp16 (PSUM accumulation fp32); elementwise fp32 on DVE/Pool.
"""
import numpy as np

import concourse.bass as bass
import concourse.mybir as mybir
import concourse.tile as tile
from concourse import bacc
from concourse import bass_utils

F32 = mybir.dt.float32
F16 = mybir.dt.float16
I16 = mybir.dt.int16
EXP = mybir.ActivationFunctionType.Exp
ADD = mybir.AluOpType.add
MULT = mybir.AluOpType.mult

B, S, D, H = 2, 2048, 1024, 16
HD = D // H          # 64
E = 256              # local features per core (4 heads)
QC = 512             # q-chunk size for the attention phase
N_QC = S // QC       # 4
N_KT = S // 128      # 16 k-tiles
KD = D // 128        # 8 contraction tiles for the projections

# Schraudolph fp16 exp: bits = round(x * 1024/ln2 + 15360 + C); the 0.125
# logit scale is folded into A. C=-44 minimizes max rel err (~3.1%); the
# constant-offset component cancels in the softmax normalization.
SCHR_A = 0.125 * 1024.0 / float(np.log(2.0))
SCHR_B = 15360.0 - 44.0
# k-tiles offloaded off the scalar engine per (qc, pr) iteration for qc >= 1
# (alternating DVE / Pool).
OFF_DVE = (3, 7, 11, 13)
OFF_POOL = ()
# dummy warmup matmuls (N=512): bridge the gap between the framework preamble
# (~6us) and the first DMA-fed matmuls (~8us) so the PE activity window that
# drives the HAM clock-gate starts as early as possible.
N_DUM = 4


def build():
    nc = bacc.Bacc("TRN2", target_bir_lowering=False, debug=False, num_devices=8)

    xT = nc.dram_tensor("xT", [D, S], F16, kind="ExternalInput").ap()
    # wkq = [wkT | wqT] concatenated host-side so one DMA per chunk loads both
    wkq = nc.dram_tensor("wkq", [D, 2 * E], F16, kind="ExternalInput").ap()
    wvT = nc.dram_tensor("wvT", [D, E], F16, kind="ExternalInput").ap()
    woT = nc.dram_tensor("woT", [E, D], F16, kind="ExternalInput").ap()
    bq2 = nc.dram_tensor("bq2", [128, 2], F32, kind="ExternalInput").ap()
    bk2 = nc.dram_tensor("bk2", [128, 2], F32, kind="ExternalInput").ap()
    bvb = nc.dram_tensor("bvb", [128, E], F32, kind="ExternalInput").ap()

    y = nc.dram_tensor("y", [S, D], F16, kind="ExternalOutput").ap()

    with tile.TileContext(nc) as tc:
        with (
            tc.tile_pool(name="pool", bufs=1) as pp,
            tc.tile_pool(name="pexp_pool", bufs=8) as pxp,
            tc.tile_pool(name="work", bufs=4) as wk,
            tc.tile_pool(name="rpool", bufs=2) as rp,
            tc.tile_pool(name="ypool", bufs=4) as yp,
            tc.tile_pool(name="ps_s", bufs=2, space="PSUM") as ps_s,
            tc.tile_pool(name="ps_av", bufs=2, space="PSUM") as ps_av,
            tc.tile_pool(name="ps_misc", bufs=2, space="PSUM") as ps_misc,
        ):
            # ---------------- persistent tiles ----------------
            woT_sb = pp.tile([128, 2, D], F16)
            bvb_sb = pp.tile([128, E], F32)
            # ones2: stationary for the fused sum-broadcast matmul:
            #   row 0  -> out partitions 64:128 (odd-head sums at col 0)
            #   row 64 -> out partitions 0:64  (even-head sums at col 64)
            ones2_sb = pp.tile([128, 128], F16)
            dum_sb = pp.tile([128, QC], F16)       # dummy warmup operand
            bq_sb = pp.tile([128, 2], F32)
            bk_sb = pp.tile([128, 2], F32)
            QT_sb = pp.tile([128, 2, S], F16)
            KT_sb = pp.tile([128, 2, S], F16)
            OT_sb = pp.tile([128, 2, S], F16)
            xT_sb = pp.tile([128, KD, S], F16)
            wkq_sb = pp.tile([128, KD, 2 * E], F16)
            wv_sb = pp.tile([128, KD, E], F16)
            # V_aug per pair (128 cols each so the matmul dst is a full
            # 128-partition AP):
            #   even head: [*, kt, 0:64]=V, col 64=1, cols 65:128=0
            #   odd head:  col 0=1, cols 1:64=0, [*, kt, 64:128]=V
            Ve_sb = [pp.tile([128, N_KT, 128], F16, name=f"ve{p}", tag=f"ve{p}")
                     for p in range(2)]
            Vo_sb = [pp.tile([128, N_KT, 128], F16, name=f"vo{p}", tag=f"vo{p}")
                     for p in range(2)]

            # ---------------- on-chip constant init + dummy PE warmup ------
            # The V_aug ones/zeros patterns and the broadcast stationary are
            # built with memsets (saves ~2.5MB of HBM traffic at warmup; the
            # input phase is DMA-bandwidth-bound). The dummy matmuls keep the
            # PE array busy from ~6us (right after the framework preamble) so
            # the HAM clock-gate reaches K=8/8 (~2.4GHz) at ~9.5us instead of
            # ~35us — the real early matmuls then run at full clock.
            nc.vector.memset(dum_sb[:], 0.0)
            nc.gpsimd.memset(ones2_sb[:], 0.0)
            nc.gpsimd.memset(ones2_sb[0:1, 64:128], 1.0)
            nc.gpsimd.memset(ones2_sb[64:65, 0:64], 1.0)
            for i in range(N_DUM):
                ps_d = ps_av.tile([128, QC], F32, tag="av")
                nc.tensor.matmul(ps_d[:], dum_sb[:, 0:128], dum_sb[:],
                                 start=True, stop=True)
            for pr in range(2):
                nc.vector.memset(Ve_sb[pr][:], 0.0)
                nc.vector.memset(Ve_sb[pr][:, :, 64:65], 1.0)
                nc.vector.memset(Vo_sb[pr][:], 0.0)
                nc.vector.memset(Vo_sb[pr][:, :, 0:1], 1.0)

            # ---------------- input DMAs ----------------
            # Per-queue DMA transfers serialize at ~1.3-3us apiece, so the
            # ~6MB of input is BALANCED across the three DMA-capable queues
            # (~2MB each), ordered first-needed-first:
            # sync: xT s0:1024; scalar: biases + wkq + xT-hi k0:4;
            # gpsimd: wv + bvb + xT-hi k4:8 + woT.
            nc.scalar.dma_start(bk_sb[:], bk2)
            nc.scalar.dma_start(bq_sb[:], bq2)
            for k in range(KD):
                nc.sync.dma_start(
                    xT_sb[:, k, 0:1024], xT[k * 128:(k + 1) * 128, 0:1024])
            for k in range(KD):
                nc.scalar.dma_start(
                    wkq_sb[:, k, :], wkq[k * 128:(k + 1) * 128, :])
            for k in range(KD):
                nc.gpsimd.dma_start(wv_sb[:, k, :], wvT[k * 128:(k + 1) * 128, :])
            nc.gpsimd.dma_start(bvb_sb[:], bvb)
            # xT-hi stays OFF the scalar queue: scalar must start the exp
            # stream right after the wkq transfers (a backlog of DMA issues
            # there stalls the first AV matmuls on exp waits).
            for k in range(KD):
                eng = nc.sync if k < 4 else nc.gpsimd
                eng.dma_start(
                    xT_sb[:, k, 1024:2048], xT[k * 128:(k + 1) * 128, 1024:2048])
            for p in range(2):
                nc.gpsimd.dma_start(woT_sb[:, p, :], woT[p * 128:(p + 1) * 128, :])

            # ---------------- emit helpers ----------------
            def proj_chain(w0, b_sb, out_sb, ch, g):
                """[128, 512] projection chunk: out_sb[:, ch, g*512:...] =
                W_ch.T @ xT[:, g-block] + b (drained on DVE). w0 selects the
                K (0) or Q (E) half of the fused wkq weights."""
                ssl = slice(g * 512, (g + 1) * 512)
                ps = ps_misc.tile([128, 512], F32, tag="misc")
                for k in range(KD):
                    nc.tensor.matmul(
                        ps[:],
                        wkq_sb[:, k, w0 + ch * 128:w0 + (ch + 1) * 128],
                        xT_sb[:, k, ssl],
                        start=(k == 0), stop=(k == KD - 1))
                nc.vector.tensor_scalar(
                    out_sb[:, ch, ssl], ps[:], b_sb[:, ch:ch + 1], None, ADD)

            def kproj(ch, g):
                proj_chain(0, bk_sb, KT_sb, ch, g)

            def qproj(qc, ch):
                proj_chain(E, bq_sb, QT_sb, ch, qc)

            def vproj(st):
                """V for s-tile st (all 4 heads), scattered+biased into the
                augmented V tiles on Pool."""
                ps = ps_misc.tile([128, 512], F32, tag="misc")
                for k in range(KD):
                    nc.tensor.matmul(
                        ps[:, 0:E],
                        xT_sb[:, k, st * 128:(st + 1) * 128],
                        wv_sb[:, k, :],
                        start=(k == 0), stop=(k == KD - 1))
                for h in range(4):
                    pr, odd = h // 2, h % 2
                    dst = (Vo_sb[pr][:, st, 64:128] if odd
                           else Ve_sb[pr][:, st, 0:64])
                    nc.vector.tensor_tensor(
                        dst, ps[:, h * 64:(h + 1) * 64],
                        bvb_sb[:, h * 64:(h + 1) * 64], ADD)

            y_tiles = {}

            def outproj_unit(st, nch, tail=False):
                """y[s-tile st, nch*512:...] = OT[:, :, ssl].T @ woT (both
                contraction chunks), drained to fp16 into a per-st staging
                tile; the nch==1 unit DMAs the full [128, 1024] row block
                (half the DMA issues = shorter tail + epilogue). In the tail
                the drains alternate DVE/ACT and the DMAs rotate queues so
                the 8 units pipeline instead of serializing."""
                ssl = slice(st * 128, (st + 1) * 128)
                psy = ps_misc.tile([128, 512], F32, tag="misc")
                for cc in range(2):
                    nc.tensor.matmul(
                        psy[:], OT_sb[:, cc, ssl],
                        woT_sb[:, cc, nch * 512:(nch + 1) * 512],
                        start=(cc == 0), stop=(cc == 1))
                if nch == 0:
                    y_tiles[st] = yp.tile([128, 1024], F16, tag="y",
                                          name=f"y_sb{st}")
                y_sb = y_tiles[st]
                if tail and nch == 1:
                    nc.scalar.copy(y_sb[:, 512:1024], psy[:])
                else:
                    nc.vector.tensor_copy(
                        y_sb[:, nch * 512:(nch + 1) * 512], psy[:])
                if nch == 1:
                    if tail:
                        deng = (nc.sync, nc.gpsimd, nc.scalar, nc.sync)[st % 4]
                    else:
                        deng = nc.sync if st % 2 == 0 else nc.gpsimd
                    deng.dma_start(y[ssl, :], y_tiles.pop(st)[:])

            def attn_iter(qc, pr, fills, last=False):
                """One (q-chunk, head-pair) attention iteration. `fills` maps
                kt -> list of emit callables sprinkled into the loop to keep
                the PE stream dense. AV matmuls run AV_LAG k-tiles behind the
                score matmuls so the PE never waits on the exp latency.
                Returns a callable that emits the PE/DVE/Pool back half of the
                normalization (scheduled as a fill in the next iteration).
                last=True shortens the exposed end-of-kernel critical path:
                the trailing exps move to DVE (the scalar queue lags), the
                sums copies split across ACT/DVE, and the normalization
                multiplies read the AV psum directly (no need to free the
                banks for a next iteration)."""
                qsl = slice(qc * QC, (qc + 1) * QC)
                av_e = ps_av.tile([128, QC], F32, tag="av")
                av_o = ps_av.tile([128, QC], F32, tag="av")
                pexps = {}
                AV_LAG = 3

                def emit_av(kt):
                    pexp = pexps.pop(kt)
                    nc.tensor.matmul(av_e[:], Ve_sb[pr][:, kt, :],
                                     pexp[:, 0, :],
                                     start=(kt == 0), stop=(kt == N_KT - 1))
                    nc.tensor.matmul(av_o[:], Vo_sb[pr][:, kt, :],
                                     pexp[:, 1, :],
                                     start=(kt == 0), stop=(kt == N_KT - 1))

                for kt in range(N_KT):
                    for f in fills.get(kt, ()):
                        f()
                    ksl = slice(kt * 128, (kt + 1) * 128)
                    # scores for the head pair: two K=64 matmuls on PE row
                    # groups 0:63 / 64:127, issued back-to-back so they run
                    # concurrently on the array.
                    ps = ps_s.tile([128, 2, QC], F32, tag="s")
                    nc.tensor.matmul(ps[:, 0, :], KT_sb[0:64, pr, ksl],
                                     QT_sb[0:64, pr, qsl])
                    nc.tensor.matmul(ps[:, 1, :], KT_sb[64:128, pr, ksl],
                                     QT_sb[64:128, pr, qsl])
                    pexp = pxp.tile([128, 2, QC], F16, tag="pexp")
                    pexps[kt] = pexp
                    offs = OFF_DVE + ((14, 15) if last else ())
                    if qc > 0 and kt in offs:
                        nc.vector.tensor_scalar(
                            pexp[:].bitcast(I16), ps[:], SCHR_A, SCHR_B,
                            MULT, ADD)
                    else:
                        nc.scalar.activation(pexp[:], ps[:], EXP, scale=0.125)
                    if kt >= AV_LAG:
                        emit_av(kt - AV_LAG)
                for kt in range(N_KT - AV_LAG, N_KT):
                    emit_av(kt)
                # normalization, front half (DVE): sums rows to SBUF fp16
                # first (unblocks the broadcast matmuls), then drain psav to
                # SBUF (frees the PSUM banks for the next iteration).
                sums16 = wk.tile([128, QC], F16, tag="sums16")
                # av_o partitions 1:64 are exact zeros (V_aug odd cols 1:64
                # are zero), so this one copy fills sums16[0:64] with
                # [sums_o, 0, 0, ...] — making the K=65 broadcast matmul
                # below safe (no uninitialized SBUF enters the array).
                if last:
                    nc.scalar.copy(sums16[0:64, :], av_o[0:64, :])
                    nc.vector.tensor_copy(sums16[64:65, :], av_e[64:65, :])
                    avsb_e = avsb_o = None
                else:
                    nc.vector.tensor_copy(sums16[0:64, :], av_o[0:64, :])
                    nc.vector.tensor_copy(sums16[64:65, :], av_e[64:65, :])
                    avsb_e = wk.tile([128, QC], F32, tag="avsb")
                    avsb_o = wk.tile([128, QC], F32, tag="avsb")
                    nc.vector.tensor_copy(avsb_e[:], av_e[:])
                    nc.vector.tensor_copy(avsb_o[:], av_o[:])

                def finish_norm(fast=False):
                    # broadcast both raw sums (partition 64 = even head,
                    # partition 0 = odd head) with ONE K=65 ones-matmul into
                    # one PSUM tile, one approx reciprocal over all 128
                    # partitions, then scale on Pool (all-SBUF operands).
                    psbc = ps_misc.tile([128, 512], F32, tag="misc")
                    nc.tensor.matmul(psbc[:], ones2_sb[0:65, :],
                                     sums16[0:65, :])
                    rec = rp.tile([128, QC], F32, tag="rec")
                    nc.vector.reciprocal_approx_fast(rec[:], psbc[:])
                    if last:
                        # read the AV psum directly — the banks are not
                        # needed again, and it removes two serial DVE copies
                        # from the end-of-kernel critical path.
                        nc.vector.tensor_tensor(
                            OT_sb[0:64, pr, qsl], av_e[0:64, :], rec[0:64, :],
                            MULT)
                        nc.vector.tensor_tensor(
                            OT_sb[64:128, pr, qsl], av_o[64:128, :],
                            rec[64:128, :], MULT)
                        return
                    eng = nc.vector if (fast or pr == 1) else nc.gpsimd
                    eng.tensor_tensor(
                        OT_sb[0:64, pr, qsl], avsb_e[0:64, :], rec[0:64, :],
                        MULT)
                    eng.tensor_tensor(
                        OT_sb[64:128, pr, qsl], avsb_o[64:128, :],
                        rec[64:128, :], MULT)

                return finish_norm

            # ---------------- fused main loop ----------------
            # Warmup phase A, k-outer: the input DMAs deliver one k-chunk of
            # xT/wkq/wv every ~1.3us per queue; iterating the CONTRACTION
            # index outermost lets every arriving chunk unlock one matmul in
            # each of 8 concurrent accumulation chains (kproj g0/g1, qproj
            # g0, vproj st0-4), keeping the PE dense from the very first
            # chunk. The chains borrow the attention pools' PSUM banks
            # (idle until ~18us).
            wu_k0 = ps_misc.tile([128, 512], F32, tag="misc")
            wu_q0 = ps_misc.tile([128, 512], F32, tag="misc")
            wu_s = [ps_s.tile([128, 2, QC], F32, tag="s", name=f"wu_s{i}")
                    for i in range(2)]
            wu_a = [ps_av.tile([128, QC], F32, tag="av", name=f"wu_a{i}")
                    for i in range(2)]
            vp_ps = [wu_s[0][:, 1, 0:E], wu_s[1][:, 0, 0:E],
                     wu_s[1][:, 1, 0:E], wu_a[0][:, 0:E], wu_a[1][:, 0:E]]
            for k in range(KD):
                st_, sp_ = (k == 0), (k == KD - 1)
                for st in range(5):
                    nc.tensor.matmul(
                        vp_ps[st], xT_sb[:, k, st * 128:(st + 1) * 128],
                        wv_sb[:, k, :], start=st_, stop=sp_)
                nc.tensor.matmul(wu_k0[:], wkq_sb[:, k, 0:128],
                                 xT_sb[:, k, 0:512], start=st_, stop=sp_)
                nc.tensor.matmul(wu_q0[:], wkq_sb[:, k, E:E + 128],
                                 xT_sb[:, k, 0:512], start=st_, stop=sp_)
                nc.tensor.matmul(wu_s[0][:, 0, :], wkq_sb[:, k, 0:128],
                                 xT_sb[:, k, 512:1024], start=st_, stop=sp_)
                if 0 < k < KD - 1:
                    # keepalive: accumulate 0 into open chains — a numeric
                    # no-op that keeps the PE's HAM activity window busy
                    # while the next k-chunk's DMA completes.
                    nc.tensor.matmul(wu_k0[:], dum_sb[:, 0:128], dum_sb[:],
                                     start=False, stop=False)
                    nc.tensor.matmul(wu_q0[:], dum_sb[:, 0:128], dum_sb[:],
                                     start=False, stop=False)
            # phase B: drains (KT g0 + QT first — scores kt0 needs them)
            nc.vector.tensor_scalar(
                KT_sb[:, 0, 0:512], wu_k0[:], bk_sb[:, 0:1], None, ADD)
            nc.vector.tensor_scalar(
                QT_sb[:, 0, 0:512], wu_q0[:], bq_sb[:, 0:1], None, ADD)
            nc.vector.tensor_scalar(
                KT_sb[:, 0, 512:1024], wu_s[0][:, 0, :], bk_sb[:, 0:1],
                None, ADD)
            for st in range(5):
                for h in range(4):
                    pr_, odd = h // 2, h % 2
                    dst = (Vo_sb[pr_][:, st, 64:128] if odd
                           else Ve_sb[pr_][:, st, 0:64])
                    nc.vector.tensor_tensor(
                        dst, vp_ps[st][:, h * 64:(h + 1) * 64],
                        bvb_sb[:, h * 64:(h + 1) * 64], ADD)
            vproj(5)

            pending_norm = None
            for qc in range(N_QC):
                for pr in range(2):
                    fills = {}
                    if pending_norm is not None:
                        # slot 3 (not 1): gives the DVE sums16 copies of the
                        # previous iteration time to finish so the broadcast
                        # matmul doesn't stall the PE queue.
                        fills.setdefault(3, []).append(pending_norm)
                    if qc == 0:
                        # JIT K/V projections. K chunk `pr` group g must
                        # precede scores kt=4g; V tile st must precede
                        # AV kt=st (prefetch distance 2 + AV lag).
                        if pr == 0:
                            for g in range(2, 4):
                                fills.setdefault(4 * (g - 1) + 2, []).append(
                                    (lambda g=g: kproj(0, g)))
                            for st in range(6, N_KT):
                                fills.setdefault(st - 6, []).append(
                                    (lambda st=st: vproj(st)))
                            fills.setdefault(12, []).append(lambda: qproj(0, 1))
                            fills.setdefault(13, []).append(lambda: kproj(1, 0))
                        else:
                            for g in range(1, 4):
                                fills.setdefault(4 * (g - 1) + 2, []).append(
                                    (lambda g=g: kproj(1, g)))
                            fills.setdefault(12, []).append(lambda: qproj(1, 0))
                    else:
                        # out-projection of the previous q-chunk: 8 units
                        # spread across the two pr iterations.
                        for i, slot in enumerate((6, 9, 12, 15)):
                            u = pr * 4 + i
                            st, nch = (qc - 1) * 4 + u // 2, u % 2
                            fills.setdefault(slot, []).append(
                                (lambda st=st, nch=nch: outproj_unit(st, nch)))
                        if pr == 0:
                            fills.setdefault(12, []).append(
                                (lambda qc=qc: qproj(qc, 1)))
                        elif qc < N_QC - 1:
                            fills.setdefault(12, []).append(
                                (lambda qc=qc: qproj(qc + 1, 0)))
                    pending_norm = attn_iter(
                        qc, pr, fills,
                        last=(qc == N_QC - 1 and pr == 1))
            pending_norm(fast=True)

            # tail: out-projection of the last q-chunk
            for u in range(8):
                st, nch = (N_QC - 1) * 4 + u // 2, u % 2
                outproj_unit(st, nch, tail=True)

    nc.compile()
    return nc


_NC_CACHE = None
last_in_maps = None


def kernel(x, Wq, bq, Wk, bk, Wv, bv, Wo, bo):
    global _NC_CACHE, last_in_maps
    x = np.asarray(x, dtype=np.float32)
    Wq, bq = np.asarray(Wq, np.float32), np.asarray(bq, np.float32)
    Wk, bk = np.asarray(Wk, np.float32), np.asarray(bk, np.float32)
    Wv, bv = np.asarray(Wv, np.float32), np.asarray(bv, np.float32)
    Wo, bo = np.asarray(Wo, np.float32), np.asarray(bo, np.float32)

    if _NC_CACHE is None:
        _NC_CACHE = build()
    nc = _NC_CACHE

    in_maps = []
    for c in range(8):
        b, f0 = c // 4, (c % 4) * E
        fs = slice(f0, f0 + E)
        in_maps.append(dict(
            xT=np.ascontiguousarray(x[b].T).astype(np.float16),
            wkq=np.ascontiguousarray(np.concatenate(
                [Wk[fs, :].T, Wq[fs, :].T], axis=1)).astype(np.float16),
            wvT=np.ascontiguousarray(Wv[fs, :].T).astype(np.float16),
            woT=np.ascontiguousarray(Wo[:, fs].T).astype(np.float16),
            bq2=np.ascontiguousarray(bq[fs].reshape(2, 128).T),
            bk2=np.ascontiguousarray(bk[fs].reshape(2, 128).T),
            bvb=np.ascontiguousarray(np.broadcast_to(bv[fs], (128, E))),
        ))

    last_in_maps = in_maps
    res = bass_utils.run_bass_kernel_spmd(nc, in_maps, core_ids=list(range(8)))

    out = np.zeros((B, S, D), np.float32)
    for c in range(8):
        out[c // 4] += res.results[c]["y"].astype(np.float32)
    out += bo
    return out



# revision 28
# speedup vs baseline: 1.0506x; 1.0002x over previous
"""Multi-head attention (B=2, S=2048, D=1024, H=16) on 8 TRN2 NeuronCores.

Sharding: core c handles batch b = c//4 and head group hg = c%4 (4 heads,
256 features f0 = hg*256). Each core computes Q/K/V projections for its
feature slice, attention for its 4 heads, and a partial output projection
y_partial = attnout @ Wo[:, f0:f0+256].T (emitted in fp16). Host sums the 4
partials per batch and adds bo.

Single fused software-pipelined loop, engineered so the tensor engine's
instruction stream is dense (TRN2 PE DVFS only reaches 2.4GHz after ~3us of
continuous execution):
 - K/V projections are JIT-streamed per k-tile-group inside q-chunk 0's
   attention loop; Q projections prefetched one (qc,pr) iteration ahead;
   out-projection of chunk qc deferred into chunk qc+1's loop. The PE
   therefore always has independent matmuls queued while softmax exp runs.
 - scores per head pair are issued back-to-back as K=64 matmuls on PE row
   groups 0:63 / 64:127 (tile_position auto-derived from base partition) so
   the two matmuls execute concurrently on the array.
 - softmax exp: scalar-engine Exp on [128, 2, 512] PSUM spans (two heads per
   instruction), with a fraction of k-tiles offloaded to DVE/Pool via a
   one-instruction fp16 Schraudolph exp (t = round(A*s + B) written as int16,
   bit-reinterpreted as fp16; constant-offset error is common-mode across k
   and cancels in the softmax normalization).
 - normalization: row sums ride along in the AV matmul (ones column in the
   augmented V); psav is drained early to SBUF (frees the PSUM bank), sums
   broadcast across partitions via two K=1 ones-matmuls into one PSUM tile,
   a single reciprocal_approx_fast over all 128 partitions, then two
   multiplies produce attnout.T in fp16.

All matmuls fp16 (PSUM accumulation fp32); elementwise fp32 on DVE/Pool.
"""
import numpy as np

import concourse.bass as bass
import concourse.mybir as mybir
import concourse.tile as tile
from concourse import bacc
from concourse import bass_utils

F32 = mybir.dt.float32
F16 = mybir.dt.float16
I16 = mybir.dt.int16
EXP = mybir.ActivationFunctionType.Exp
ADD = mybir.AluOpType.add
MULT = mybir.AluOpType.mult

B, S, D, H = 2, 2048, 1024, 16
HD = D // H          # 64
E = 256              # local features per core (4 heads)
QC = 512             # q-chunk size for the attention phase
N_QC = S // QC       # 4
N_KT = S // 128      # 16 k-tiles
KD = D // 128        # 8 contraction tiles for the projections

# Schraudolph fp16 exp: bits = round(x * 1024/ln2 + 15360 + C); the 0.125
# logit scale is folded into A. C=-44 minimizes max rel err (~3.1%); the
# constant-offset component cancels in the softmax normalization.
SCHR_A = 0.125 * 1024.0 / float(np.log(2.0))
SCHR_B = 15360.0 - 44.0
# k-tiles offloaded off the scalar engine per (qc, pr) iteration for qc >= 1
# (alternating DVE / Pool).
OFF_DVE = (3, 7, 11, 13)
OFF_POOL = ()
# dummy warmup matmuls (N=512): bridge the gap between the framework preamble
# (~6us) and the first DMA-fed matmuls (~8us) so the PE activity window that
# drives the HAM clock-gate starts as early as possible.
N_DUM = 4


def build():
    nc = bacc.Bacc("TRN2", target_bir_lowering=False, debug=False, num_devices=8)

    xT = nc.dram_tensor("xT", [D, S], F16, kind="ExternalInput").ap()
    # wkq = [wkT | wqT] concatenated host-side so one DMA per chunk loads both
    wkq = nc.dram_tensor("wkq", [D, 2 * E], F16, kind="ExternalInput").ap()
    wvT = nc.dram_tensor("wvT", [D, E], F16, kind="ExternalInput").ap()
    woT = nc.dram_tensor("woT", [E, D], F16, kind="ExternalInput").ap()
    bq2 = nc.dram_tensor("bq2", [128, 2], F32, kind="ExternalInput").ap()
    bk2 = nc.dram_tensor("bk2", [128, 2], F32, kind="ExternalInput").ap()
    bvb = nc.dram_tensor("bvb", [128, E], F32, kind="ExternalInput").ap()

    y = nc.dram_tensor("y", [S, D], F16, kind="ExternalOutput").ap()

    with tile.TileContext(nc) as tc:
        with (
            tc.tile_pool(name="pool", bufs=1) as pp,
            tc.tile_pool(name="pexp_pool", bufs=8) as pxp,
            tc.tile_pool(name="work", bufs=4) as wk,
            tc.tile_pool(name="rpool", bufs=2) as rp,
            tc.tile_pool(name="ypool", bufs=4) as yp,
            tc.tile_pool(name="ps_s", bufs=2, space="PSUM") as ps_s,
            tc.tile_pool(name="ps_av", bufs=2, space="PSUM") as ps_av,
            tc.tile_pool(name="ps_misc", bufs=2, space="PSUM") as ps_misc,
        ):
            # ---------------- persistent tiles ----------------
            woT_sb = pp.tile([128, 2, D], F16)
            bvb_sb = pp.tile([128, E], F32)
            # ones2: stationary for the fused sum-broadcast matmul:
            #   row 0  -> out partitions 64:128 (odd-head sums at col 0)
            #   row 64 -> out partitions 0:64  (even-head sums at col 64)
            ones2_sb = pp.tile([128, 128], F16)
            dum_sb = pp.tile([128, QC], F16)       # dummy warmup operand
            bq_sb = pp.tile([128, 2], F32)
            bk_sb = pp.tile([128, 2], F32)
            QT_sb = pp.tile([128, 2, S], F16)
            KT_sb = pp.tile([128, 2, S], F16)
            OT_sb = pp.tile([128, 2, S], F16)
            xT_sb = pp.tile([128, KD, S], F16)
            wkq_sb = pp.tile([128, KD, 2 * E], F16)
            wv_sb = pp.tile([128, KD, E], F16)
            # V_aug per pair (128 cols each so the matmul dst is a full
            # 128-partition AP):
            #   even head: [*, kt, 0:64]=V, col 64=1, cols 65:128=0
            #   odd head:  col 0=1, cols 1:64=0, [*, kt, 64:128]=V
            Ve_sb = [pp.tile([128, N_KT, 128], F16, name=f"ve{p}", tag=f"ve{p}")
                     for p in range(2)]
            Vo_sb = [pp.tile([128, N_KT, 128], F16, name=f"vo{p}", tag=f"vo{p}")
                     for p in range(2)]

            # ---------------- on-chip constant init + dummy PE warmup ------
            # The V_aug ones/zeros patterns and the broadcast stationary are
            # built with memsets (saves ~2.5MB of HBM traffic at warmup; the
            # input phase is DMA-bandwidth-bound). The dummy matmuls keep the
            # PE array busy from ~6us (right after the framework preamble) so
            # the HAM clock-gate reaches K=8/8 (~2.4GHz) at ~9.5us instead of
            # ~35us — the real early matmuls then run at full clock.
            nc.vector.memset(dum_sb[:], 0.0)
            nc.gpsimd.memset(ones2_sb[:], 0.0)
            nc.gpsimd.memset(ones2_sb[0:1, 64:128], 1.0)
            nc.gpsimd.memset(ones2_sb[64:65, 0:64], 1.0)
            for i in range(N_DUM):
                ps_d = ps_av.tile([128, QC], F32, tag="av")
                nc.tensor.matmul(ps_d[:], dum_sb[:, 0:128], dum_sb[:],
                                 start=True, stop=True)
            for pr in range(2):
                nc.vector.memset(Ve_sb[pr][:], 0.0)
                nc.vector.memset(Ve_sb[pr][:, :, 64:65], 1.0)
                nc.vector.memset(Vo_sb[pr][:], 0.0)
                nc.vector.memset(Vo_sb[pr][:, :, 0:1], 1.0)

            # ---------------- input DMAs ----------------
            # Per-queue DMA transfers serialize at ~1.3-3us apiece, so the
            # ~6MB of input is BALANCED across the three DMA-capable queues
            # (~2MB each), ordered first-needed-first:
            # sync: xT s0:1024; scalar: biases + wkq + xT-hi k0:4;
            # gpsimd: wv + bvb + xT-hi k4:8 + woT.
            nc.scalar.dma_start(bk_sb[:], bk2)
            nc.scalar.dma_start(bq_sb[:], bq2)
            for k in range(KD):
                nc.sync.dma_start(
                    xT_sb[:, k, 0:1024], xT[k * 128:(k + 1) * 128, 0:1024])
            for k in range(KD):
                nc.scalar.dma_start(
                    wkq_sb[:, k, :], wkq[k * 128:(k + 1) * 128, :])
            for k in range(KD):
                nc.gpsimd.dma_start(wv_sb[:, k, :], wvT[k * 128:(k + 1) * 128, :])
            nc.gpsimd.dma_start(bvb_sb[:], bvb)
            # xT-hi stays OFF the scalar queue: scalar must start the exp
            # stream right after the wkq transfers (a backlog of DMA issues
            # there stalls the first AV matmuls on exp waits).
            for k in range(KD):
                eng = nc.sync if k < 4 else nc.gpsimd
                eng.dma_start(
                    xT_sb[:, k, 1024:2048], xT[k * 128:(k + 1) * 128, 1024:2048])
            for p in range(2):
                nc.gpsimd.dma_start(woT_sb[:, p, :], woT[p * 128:(p + 1) * 128, :])

            # ---------------- emit helpers ----------------
            def proj_chain(w0, b_sb, out_sb, ch, g):
                """[128, 512] projection chunk: out_sb[:, ch, g*512:...] =
                W_ch.T @ xT[:, g-block] + b (drained on DVE). w0 selects the
                K (0) or Q (E) half of the fused wkq weights."""
                ssl = slice(g * 512, (g + 1) * 512)
                ps = ps_misc.tile([128, 512], F32, tag="misc")
                for k in range(KD):
                    nc.tensor.matmul(
                        ps[:],
                        wkq_sb[:, k, w0 + ch * 128:w0 + (ch + 1) * 128],
                        xT_sb[:, k, ssl],
                        start=(k == 0), stop=(k == KD - 1))
                nc.vector.tensor_scalar(
                    out_sb[:, ch, ssl], ps[:], b_sb[:, ch:ch + 1], None, ADD)

            def kproj(ch, g):
                proj_chain(0, bk_sb, KT_sb, ch, g)

            def qproj(qc, ch):
                proj_chain(E, bq_sb, QT_sb, ch, qc)

            def vproj(st):
                """V for s-tile st (all 4 heads), scattered+biased into the
                augmented V tiles on Pool."""
                ps = ps_misc.tile([128, 512], F32, tag="misc")
                for k in range(KD):
                    nc.tensor.matmul(
                        ps[:, 0:E],
                        xT_sb[:, k, st * 128:(st + 1) * 128],
                        wv_sb[:, k, :],
                        start=(k == 0), stop=(k == KD - 1))
                for h in range(4):
                    pr, odd = h // 2, h % 2
                    dst = (Vo_sb[pr][:, st, 64:128] if odd
                           else Ve_sb[pr][:, st, 0:64])
                    nc.vector.tensor_tensor(
                        dst, ps[:, h * 64:(h + 1) * 64],
                        bvb_sb[:, h * 64:(h + 1) * 64], ADD)

            y_tiles = {}

            def outproj_unit(st, nch, tail=False):
                """y[s-tile st, nch*512:...] = OT[:, :, ssl].T @ woT (both
                contraction chunks), drained to fp16 into a per-st staging
                tile; the nch==1 unit DMAs the full [128, 1024] row block
                (half the DMA issues = shorter tail + epilogue). In the tail
                the drains alternate DVE/ACT and the DMAs rotate queues so
                the 8 units pipeline instead of serializing."""
                ssl = slice(st * 128, (st + 1) * 128)
                psy = ps_misc.tile([128, 512], F32, tag="misc")
                for cc in range(2):
                    nc.tensor.matmul(
                        psy[:], OT_sb[:, cc, ssl],
                        woT_sb[:, cc, nch * 512:(nch + 1) * 512],
                        start=(cc == 0), stop=(cc == 1))
                if nch == 0:
                    y_tiles[st] = yp.tile([128, 1024], F16, tag="y",
                                          name=f"y_sb{st}")
                y_sb = y_tiles[st]
                if tail and nch == 1:
                    nc.scalar.copy(y_sb[:, 512:1024], psy[:])
                else:
                    nc.vector.tensor_copy(
                        y_sb[:, nch * 512:(nch + 1) * 512], psy[:])
                if nch == 1:
                    if tail:
                        deng = (nc.sync, nc.gpsimd, nc.scalar, nc.sync)[st % 4]
                    else:
                        deng = nc.sync if st % 2 == 0 else nc.gpsimd
                    deng.dma_start(y[ssl, :], y_tiles.pop(st)[:])

            def attn_iter(qc, pr, fills, last=False):
                """One (q-chunk, head-pair) attention iteration. `fills` maps
                kt -> list of emit callables sprinkled into the loop to keep
                the PE stream dense. AV matmuls run AV_LAG k-tiles behind the
                score matmuls so the PE never waits on the exp latency.
                Returns a callable that emits the PE/DVE/Pool back half of the
                normalization (scheduled as a fill in the next iteration).
                last=True shortens the exposed end-of-kernel critical path:
                the trailing exps move to DVE (the scalar queue lags), the
                sums copies split across ACT/DVE, and the normalization
                multiplies read the AV psum directly (no need to free the
                banks for a next iteration)."""
                qsl = slice(qc * QC, (qc + 1) * QC)
                av_e = ps_av.tile([128, QC], F32, tag="av")
                av_o = ps_av.tile([128, QC], F32, tag="av")
                pexps = {}
                AV_LAG = 3

                def emit_av(kt):
                    pexp = pexps.pop(kt)
                    nc.tensor.matmul(av_e[:], Ve_sb[pr][:, kt, :],
                                     pexp[:, 0, :],
                                     start=(kt == 0), stop=(kt == N_KT - 1))
                    nc.tensor.matmul(av_o[:], Vo_sb[pr][:, kt, :],
                                     pexp[:, 1, :],
                                     start=(kt == 0), stop=(kt == N_KT - 1))

                for kt in range(N_KT):
                    for f in fills.get(kt, ()):
                        f()
                    ksl = slice(kt * 128, (kt + 1) * 128)
                    # scores for the head pair: two K=64 matmuls on PE row
                    # groups 0:63 / 64:127, issued back-to-back so they run
                    # concurrently on the array.
                    ps = ps_s.tile([128, 2, QC], F32, tag="s")
                    nc.tensor.matmul(ps[:, 0, :], KT_sb[0:64, pr, ksl],
                                     QT_sb[0:64, pr, qsl])
                    nc.tensor.matmul(ps[:, 1, :], KT_sb[64:128, pr, ksl],
                                     QT_sb[64:128, pr, qsl])
                    pexp = pxp.tile([128, 2, QC], F16, tag="pexp")
                    pexps[kt] = pexp
                    offs = OFF_DVE + ((14, 15) if last else ())
                    if qc > 0 and kt in offs:
                        nc.vector.tensor_scalar(
                            pexp[:].bitcast(I16), ps[:], SCHR_A, SCHR_B,
                            MULT, ADD)
                    else:
                        nc.scalar.activation(pexp[:], ps[:], EXP, scale=0.125)
                    if kt >= AV_LAG:
                        emit_av(kt - AV_LAG)
                for kt in range(N_KT - AV_LAG, N_KT):
                    emit_av(kt)
                # normalization, front half (DVE): sums rows to SBUF fp16
                # first (unblocks the broadcast matmuls), then drain psav to
                # SBUF (frees the PSUM banks for the next iteration).
                sums16 = wk.tile([128, QC], F16, tag="sums16")
                # av_o partitions 1:64 are exact zeros (V_aug odd cols 1:64
                # are zero), so this one copy fills sums16[0:64] with
                # [sums_o, 0, 0, ...] — making the K=65 broadcast matmul
                # below safe (no uninitialized SBUF enters the array).
                if last:
                    nc.scalar.copy(sums16[0:64, :], av_o[0:64, :])
                    nc.vector.tensor_copy(sums16[64:65, :], av_e[64:65, :])
                    avsb_e = avsb_o = None
                else:
                    nc.vector.tensor_copy(sums16[0:64, :], av_o[0:64, :])
                    nc.vector.tensor_copy(sums16[64:65, :], av_e[64:65, :])
                    avsb_e = wk.tile([128, QC], F32, tag="avsb")
                    avsb_o = wk.tile([128, QC], F32, tag="avsb")
                    nc.vector.tensor_copy(avsb_e[:], av_e[:])
                    nc.vector.tensor_copy(avsb_o[:], av_o[:])

                def finish_norm(fast=False):
                    # broadcast both raw sums (partition 64 = even head,
                    # partition 0 = odd head) with ONE K=65 ones-matmul into
                    # one PSUM tile, one approx reciprocal over all 128
                    # partitions, then scale on Pool (all-SBUF operands).
                    psbc = ps_misc.tile([128, 512], F32, tag="misc")
                    nc.tensor.matmul(psbc[:], ones2_sb[0:65, :],
                                     sums16[0:65, :])
                    rec = rp.tile([128, QC], F32, tag="rec")
                    nc.vector.reciprocal_approx_fast(rec[:], psbc[:])
                    if last:
                        # read the AV psum directly — the banks are not
                        # needed again, and it removes two serial DVE copies
                        # from the end-of-kernel critical path.
                        nc.vector.tensor_tensor(
                            OT_sb[0:64, pr, qsl], av_e[0:64, :], rec[0:64, :],
                            MULT)
                        nc.vector.tensor_tensor(
                            OT_sb[64:128, pr, qsl], av_o[64:128, :],
                            rec[64:128, :], MULT)
                        return
                    eng = nc.vector if (fast or pr == 1) else nc.gpsimd
                    eng.tensor_tensor(
                        OT_sb[0:64, pr, qsl], avsb_e[0:64, :], rec[0:64, :],
                        MULT)
                    eng.tensor_tensor(
                        OT_sb[64:128, pr, qsl], avsb_o[64:128, :],
                        rec[64:128, :], MULT)

                return finish_norm

            # ---------------- fused main loop ----------------
            # Warmup phase A, k-outer: the input DMAs deliver one k-chunk of
            # xT/wkq/wv every ~1.3us per queue; iterating the CONTRACTION
            # index outermost lets every arriving chunk unlock one matmul in
            # each of 8 concurrent accumulation chains (kproj g0/g1, qproj
            # g0, vproj st0-4), keeping the PE dense from the very first
            # chunk. The chains borrow the attention pools' PSUM banks
            # (idle until ~18us).
            wu_k0 = ps_misc.tile([128, 512], F32, tag="misc")
            wu_q0 = ps_misc.tile([128, 512], F32, tag="misc")
            wu_s = [ps_s.tile([128, 2, QC], F32, tag="s", name=f"wu_s{i}")
                    for i in range(2)]
            wu_a = [ps_av.tile([128, QC], F32, tag="av", name=f"wu_a{i}")
                    for i in range(2)]
            vp_ps = [wu_s[0][:, 1, 0:E], wu_s[1][:, 0, 0:E],
                     wu_s[1][:, 1, 0:E], wu_a[0][:, 0:E], wu_a[1][:, 0:E]]
            def vp_group(k):
                st_, sp_ = (k == 0), (k == KD - 1)
                for st in range(5):
                    nc.tensor.matmul(
                        vp_ps[st], xT_sb[:, k, st * 128:(st + 1) * 128],
                        wv_sb[:, k, :], start=st_, stop=sp_)

            def kq_group(k):
                st_, sp_ = (k == 0), (k == KD - 1)
                nc.tensor.matmul(wu_k0[:], wkq_sb[:, k, 0:128],
                                 xT_sb[:, k, 0:512], start=st_, stop=sp_)
                nc.tensor.matmul(wu_q0[:], wkq_sb[:, k, E:E + 128],
                                 xT_sb[:, k, 0:512], start=st_, stop=sp_)
                nc.tensor.matmul(wu_s[0][:, 0, :], wkq_sb[:, k, 0:128],
                                 xT_sb[:, k, 512:1024], start=st_, stop=sp_)

            # The vp groups' inputs (xT on sync, wv on gpsimd) land ~1 chunk
            # ahead of the kq groups' wkq (scalar queue, behind the biases),
            # so interleaving vp ahead of kq keeps every group's semaphores
            # already satisfied when the in-order PE queue reaches it.
            vp_group(0)
            vp_group(1)
            for k in range(6):
                kq_group(k)
                vp_group(k + 2)
            kq_group(6)
            kq_group(7)
            # phase B: drains (KT g0 + QT first — scores kt0 needs them)
            nc.vector.tensor_scalar(
                KT_sb[:, 0, 0:512], wu_k0[:], bk_sb[:, 0:1], None, ADD)
            nc.vector.tensor_scalar(
                QT_sb[:, 0, 0:512], wu_q0[:], bq_sb[:, 0:1], None, ADD)
            nc.vector.tensor_scalar(
                KT_sb[:, 0, 512:1024], wu_s[0][:, 0, :], bk_sb[:, 0:1],
                None, ADD)
            for st in range(5):
                for h in range(4):
                    pr_, odd = h // 2, h % 2
                    dst = (Vo_sb[pr_][:, st, 64:128] if odd
                           else Ve_sb[pr_][:, st, 0:64])
                    nc.vector.tensor_tensor(
                        dst, vp_ps[st][:, h * 64:(h + 1) * 64],
                        bvb_sb[:, h * 64:(h + 1) * 64], ADD)
            vproj(5)

            pending_norm = None
            for qc in range(N_QC):
                for pr in range(2):
                    fills = {}
                    if pending_norm is not None:
                        # slot 3 (not 1): gives the DVE sums16 copies of the
                        # previous iteration time to finish so the broadcast
                        # matmul doesn't stall the PE queue.
                        fills.setdefault(3, []).append(pending_norm)
                    if qc == 0:
                        # JIT K/V projections. K chunk `pr` group g must
                        # precede scores kt=4g; V tile st must precede
                        # AV kt=st (prefetch distance 2 + AV lag).
                        if pr == 0:
                            for g in range(2, 4):
                                fills.setdefault(4 * (g - 1) + 2, []).append(
                                    (lambda g=g: kproj(0, g)))
                            for st in range(6, N_KT):
                                fills.setdefault(st - 6, []).append(
                                    (lambda st=st: vproj(st)))
                            fills.setdefault(12, []).append(lambda: qproj(0, 1))
                            fills.setdefault(13, []).append(lambda: kproj(1, 0))
                        else:
                            # kproj(1,1) at slot 0 fills the iteration-
                            # boundary bubble (the PE has little ready work
                            # while the previous norm's DVE copies run).
                            for g, slot in ((1, 0), (2, 6), (3, 10)):
                                fills.setdefault(slot, []).append(
                                    (lambda g=g: kproj(1, g)))
                            fills.setdefault(12, []).append(lambda: qproj(1, 0))
                    else:
                        # out-projection of the previous q-chunk: 8 units
                        # spread across the two pr iterations. pr==1 can
                        # start at slot 0 (its OT inputs were finalized at
                        # slot 3 of the pr==0 iteration) — covering the
                        # boundary bubble; pr==0 instead front-loads the
                        # qproj, since its outproj needs this iteration's
                        # pending_norm (slot 3) first.
                        slots = (5, 8, 11, 14) if pr == 0 else (0, 5, 8, 11)
                        for i, slot in enumerate(slots):
                            u = pr * 4 + i
                            st, nch = (qc - 1) * 4 + u // 2, u % 2
                            fills.setdefault(slot, []).append(
                                (lambda st=st, nch=nch: outproj_unit(st, nch)))
                        if pr == 0:
                            fills.setdefault(0, []).append(
                                (lambda qc=qc: qproj(qc, 1)))
                        elif qc < N_QC - 1:
                            fills.setdefault(14, []).append(
                                (lambda qc=qc: qproj(qc + 1, 0)))
                    pending_norm = attn_iter(
                        qc, pr, fills,
                        last=(qc == N_QC - 1 and pr == 1))
            # endgame: zero-matmul keepalives bridge the final norm chain's
            # DVE latency so the HAM clock-gate stays at full speed for the
            # tail out-projection (the score-psum slots are free by now).
            ka1 = ps_s.tile([128, 2, QC], F32, tag="s")
            for _ in range(4):
                nc.tensor.matmul(ka1[:, 0, :], dum_sb[:, 0:128], dum_sb[:],
                                 start=True, stop=True)
            pending_norm(fast=True)
            ka2 = ps_s.tile([128, 2, QC], F32, tag="s")
            for _ in range(4):
                nc.tensor.matmul(ka2[:, 0, :], dum_sb[:, 0:128], dum_sb[:],
                                 start=True, stop=True)

            # tail: out-projection of the last q-chunk
            for u in range(8):
                st, nch = (N_QC - 1) * 4 + u // 2, u % 2
                outproj_unit(st, nch, tail=True)

    nc.compile()
    return nc


_NC_CACHE = None
last_in_maps = None


def kernel(x, Wq, bq, Wk, bk, Wv, bv, Wo, bo):
    global _NC_CACHE, last_in_maps
    x = np.asarray(x, dtype=np.float32)
    Wq, bq = np.asarray(Wq, np.float32), np.asarray(bq, np.float32)
    Wk, bk = np.asarray(Wk, np.float32), np.asarray(bk, np.float32)
    Wv, bv = np.asarray(Wv, np.float32), np.asarray(bv, np.float32)
    Wo, bo = np.asarray(Wo, np.float32), np.asarray(bo, np.float32)

    if _NC_CACHE is None:
        _NC_CACHE = build()
    nc = _NC_CACHE

    in_maps = []
    for c in range(8):
        b, f0 = c // 4, (c % 4) * E
        fs = slice(f0, f0 + E)
        in_maps.append(dict(
            xT=np.ascontiguousarray(x[b].T).astype(np.float16),
            wkq=np.ascontiguousarray(np.concatenate(
                [Wk[fs, :].T, Wq[fs, :].T], axis=1)).astype(np.float16),
            wvT=np.ascontiguousarray(Wv[fs, :].T).astype(np.float16),
            woT=np.ascontiguousarray(Wo[:, fs].T).astype(np.float16),
            bq2=np.ascontiguousarray(bq[fs].reshape(2, 128).T),
            bk2=np.ascontiguousarray(bk[fs].reshape(2, 128).T),
            bvb=np.ascontiguousarray(np.broadcast_to(bv[fs], (128, E))),
        ))

    last_in_maps = in_maps
    res = bass_utils.run_bass_kernel_spmd(nc, in_maps, core_ids=list(range(8)))

    out = np.zeros((B, S, D), np.float32)
    for c in range(8):
        out[c // 4] += res.results[c]["y"].astype(np.float32)
    out += bo
    return out

